# revision 1
# baseline (speedup 1.0000x reference)
"""Multi-head attention block kernel for Trainium2, sharded over 8 NeuronCores.

Sharding: batch (4) x head-group (2 groups of 8 heads) -> 8 cores.
Each core computes, for one batch b and one half of the heads:
  qh/kh/vh projections (columns of w_q/w_k/w_v for its heads),
  causal attention for its 8 heads, and a partial output projection
  (rows of w_o^T for its heads).  Host sums the two partial outputs per
  batch and transposes back.

On-chip layout is feature-major ("transposed"): activations live as
[feature, seq] so every matmul contraction dim is on partitions and no
on-chip transposes are needed.  Host pre-transposes q/k/v and the
weight slices, and post-transposes the output.

Matmuls run in bf16 (fp32 matmul is 4x slower on TRN2); accumulation is
fp32 in PSUM.  Softmax denominators come for free from an extra ones
column appended to each V tile (row 64 of the attn@V accumulator is the
sum of exp scores).
"""

import sys

sys.path.insert(0, "/opt/trn_rl_repo")

import numpy as np
import ml_dtypes

import concourse.bacc as bacc
import concourse.mybir as mybir
import concourse.tile as tile
from concourse import bass_utils

B = 4
S = 2048
E = 1024
HEADS = 16
D = 64
H = 8            # heads per core
F = H * D        # 512 local head features
P = 128
ET = E // P      # 8 e-tiles
FT = F // P      # 4 f-tiles
ST = S // P      # 16 s-tiles
QC = 512         # q-chunk width
NQC = S // QC    # 4 q-chunks
KT_PER_QC = QC // P  # 4 k-tiles per q-chunk

BF16 = mybir.dt.bfloat16
F32 = mybir.dt.float32
NPBF16 = ml_dtypes.bfloat16


def build_nc(causal: bool, niter: int | None = None, phases=(1, 2, 3), no_norm=False, no_exp=False,
             p1_wide=4, p3_wide=4, xtlag=2, sc_bufs=0, ps_bufs=4, at_bufs=12, old_p2=False):
    """Build the per-core Bass program.  If niter is given, wrap the body in a
    For_i timing loop (used by test.py to measure HW time)."""
    nc = bacc.Bacc("TRN2", target_bir_lowering=False, debug=False,
                   enable_asserts=True, num_devices=8)

    qT = nc.dram_tensor("qT", [E, S], BF16, kind="ExternalInput").ap()
    kT = nc.dram_tensor("kT", [E, S], BF16, kind="ExternalInput").ap()
    vT = nc.dram_tensor("vT", [E, S], BF16, kind="ExternalInput").ap()
    wqT = nc.dram_tensor("wqT", [E, F], BF16, kind="ExternalInput").ap()
    wkT = nc.dram_tensor("wkT", [E, F], BF16, kind="ExternalInput").ap()
    wvT = nc.dram_tensor("wvT", [E, F], BF16, kind="ExternalInput").ap()
    woT = nc.dram_tensor("woT", [F, E], BF16, kind="ExternalInput").ap()
    stair = nc.dram_tensor("stair", [P, 2 * QC], BF16, kind="ExternalInput").ap()
    if not causal:
        maskT = nc.dram_tensor("maskT", [S, S], BF16, kind="ExternalInput").ap()
    outT = nc.dram_tensor("outT", [E, S], F32, kind="ExternalOutput").ap()

    qT3 = qT.rearrange("(o p) s -> p o s", p=P)
    kT3 = kT.rearrange("(o p) s -> p o s", p=P)
    vT3 = vT.rearrange("(o p) s -> p o s", p=P)
    if not causal:
        maskT3 = maskT.rearrange("(o p) s -> p o s", p=P)

    with tile.TileContext(nc) as tc:
        import contextlib
        with contextlib.ExitStack() as ctx:
            persist = ctx.enter_context(tc.tile_pool(name="persist", bufs=1))
            streams = ctx.enter_context(tc.tile_pool(name="streams", bufs=6))
            attnp = ctx.enter_context(tc.tile_pool(name="attnp", bufs=at_bufs))
            smalls = ctx.enter_context(tc.tile_pool(name="smalls", bufs=3))
            ps1 = ctx.enter_context(tc.tile_pool(name="ps1", bufs=ps_bufs, space="PSUM"))
            if sc_bufs:
                ps_sc = ctx.enter_context(tc.tile_pool(name="ps_sc", bufs=sc_bufs, space="PSUM"))
            ps_xt = ctx.enter_context(tc.tile_pool(name="ps_xt", bufs=1, space="PSUM"))
            def sc_tile():
                if sc_bufs:
                    return ps_sc.tile([P, QC], F32, tag="sc", name="scp")
                return ps1.tile([P, QC], F32, tag="ps", name="scp")

            # Weights + constants: loaded once, outside the timing loop.
            wq_sb = persist.tile([P, ET, F], BF16, tag="wq")
            wk_sb = persist.tile([P, ET, F], BF16, tag="wk")
            wv_sb = persist.tile([P, ET, F], BF16, tag="wv")
            wo_sb = persist.tile([P, FT, E], BF16, tag="wo")
            stair_sb = persist.tile([P, 2 * QC], BF16, tag="stair")
            nc.sync.dma_start(wq_sb[:], wqT.rearrange("(o p) f -> p o f", p=P))
            nc.sync.dma_start(wk_sb[:], wkT.rearrange("(o p) f -> p o f", p=P))
            nc.sync.dma_start(wv_sb[:], wvT.rearrange("(o p) f -> p o f", p=P))
            nc.sync.dma_start(wo_sb[:], woT.rearrange("(o p) e -> p o e", p=P))
            nc.sync.dma_start(stair_sb[:], stair[:])

            # Persistent activations (bf16): projections and attention outputs.
            qh_sb = persist.tile([P, FT, S], BF16, tag="qh")    # [f, ft, s]
            kh_sb = persist.tile([P, FT, S], BF16, tag="kh")
            vh_sb = persist.tile([P, ST, H, D + 1], BF16, tag="vh")  # ones col at d=64
            xts_sb = persist.tile([P, FT, S], BF16, tag="xts")

            def body():
                run1 = 1 in phases
                run2 = 2 in phases
                run3 = 3 in phases
                if not run1:
                    nc.vector.memset(qh_sb[:, :, 0:1], 0.5)
                    nc.vector.memset(kh_sb[:, :, 0:1], 0.5)
                    nc.vector.memset(vh_sb[:, :, :, 0:1], 0.5)
                if not run2 and run3:
                    nc.vector.memset(xts_sb[:, :, 0:1], 0.5)
                # ---- Phase 1a: q/k projections -> qh/kh (feature-major) ----
                # Weight-stationary: for each (ft, et) weight tile, stream all
                # 4 s-chunks into 4 accumulating PSUMs so LDWEIGHTS happens
                # once per 4 matmuls.
                for src3, w_sb, dst in ((qT3, wq_sb, qh_sb), (kT3, wk_sb, kh_sb)) if run1 else ():
                    xcs = []
                    for sc in range(NQC):
                        xc = streams.tile([P, ET, QC], BF16, tag="xc")
                        nc.sync.dma_start(xc[:], src3[:, :, sc * QC:(sc + 1) * QC])
                        xcs.append(xc)
                    for ft in range(FT):
                        if p1_wide > 1:
                            for g0 in range(0, NQC, p1_wide):
                                gs = list(range(g0, min(NQC, g0 + p1_wide)))
                                psums = [ps1.tile([P, QC], F32, tag="ps", name=f"pp{sc}")
                                         for sc in gs]
                                for et in range(ET):
                                    for i, sc in enumerate(gs):
                                        nc.tensor.matmul(
                                            psums[i][:],
                                            w_sb[:, et, ft * P:(ft + 1) * P],
                                            xcs[sc][:, et, :],
                                            start=(et == 0), stop=(et == ET - 1))
                                for i, sc in enumerate(gs):
                                    nc.vector.tensor_copy(
                                        dst[:, ft, sc * QC:(sc + 1) * QC], psums[i][:])
                        else:
                            for sc in range(NQC):
                                psum = ps1.tile([P, QC], F32, tag="ps", name="pp")
                                for et in range(ET):
                                    nc.tensor.matmul(
                                        psum[:],
                                        w_sb[:, et, ft * P:(ft + 1) * P],
                                        xcs[sc][:, et, :],
                                        start=(et == 0), stop=(et == ET - 1))
                                nc.vector.tensor_copy(
                                    dst[:, ft, sc * QC:(sc + 1) * QC], psum[:])

                # ---- Phase 1b: v projection -> vh (seq-major) + ones column ----
                nc.vector.memset(vh_sb[:, :, :, D:D + 1], 1.0)
                for sc in range(NQC) if run1 else ():
                    xc = streams.tile([P, ET, QC], BF16, tag="xc")
                    nc.sync.dma_start(xc[:], vT3[:, :, sc * QC:(sc + 1) * QC])
                    for si in range(KT_PER_QC):
                        st = sc * KT_PER_QC + si
                        psum = ps1.tile([P, QC], F32, tag="ps")
                        for et in range(ET):
                            nc.tensor.matmul(
                                psum[:],
                                xc[:, et, si * P:(si + 1) * P],
                                wv_sb[:, et, :],
                                start=(et == 0), stop=(et == ET - 1))
                        nc.vector.tensor_copy(
                            vh_sb[:, st, :, 0:D],
                            psum[:].rearrange("p (h d) -> p h d", h=H))

                # ---- Phase 2: attention ----
                # Causal path: kt-outer so the stationary operands (K tile for
                # scores, V tile for attn@V) are each loaded once per (h, kt)
                # and streamed over all valid q-chunks (LDWEIGHTS amortization;
                # weight switches cost ~250ns on PE).  Needs one xt accumulator
                # per q-chunk (4 PSUM banks).
                def normalize(xt_psum, h, qc):
                    ft, fo = h // 2, (h % 2) * D
                    if no_norm:
                        nc.vector.tensor_copy(
                            xts_sb[fo:fo + D, ft, qc * QC:(qc + 1) * QC],
                            xt_psum[0:D, :])
                    else:
                        recip = smalls.tile([1, QC], F32, tag="recip")
                        nc.vector.reciprocal(recip[:], xt_psum[D:D + 1, :])
                        rb = smalls.tile([D, QC], F32, tag="rb")
                        nc.gpsimd.partition_broadcast(rb[:], recip[0:1, :])
                        nc.vector.tensor_mul(
                            xts_sb[fo:fo + D, ft, qc * QC:(qc + 1) * QC],
                            xt_psum[0:D, :], rb[:])

                def emit_exp(at, sc_psum, kt, qc, mc):
                    if no_exp:
                        nc.vector.tensor_copy(at[:], sc_psum[:])
                    else:
                        nc.scalar.activation(at[:], sc_psum[:],
                                             mybir.ActivationFunctionType.Exp,
                                             scale=0.125)
                    if causal:
                        if kt // KT_PER_QC == qc:
                            off = kt * P - qc * QC
                            nc.vector.tensor_mul(
                                at[:], at[:], stair_sb[:, QC - off:2 * QC - off])
                    else:
                        nc.vector.tensor_mul(at[:], at[:], mc[:, kt, :])

                if run2 and causal and not old_p2:
                    for h in range(H):
                        ft, fo = h // 2, (h % 2) * D
                        xt_psums = [ps_xt.tile([D + 1, QC], F32, tag=f"xt{qc}", name=f"xt{qc}")
                                    for qc in range(NQC)]
                        pend = []   # [(kt, qc, at)] generations awaiting attn@V

                        def flush_xt(gen):
                            for kt, qc, at in gen:
                                nc.tensor.matmul(
                                    xt_psums[qc][:],
                                    vh_sb[:, kt, h, :],
                                    at[:],
                                    start=(kt == 0),
                                    stop=(kt == (qc + 1) * KT_PER_QC - 1))

                        XTLAG = xtlag
                        for kt in range(ST):
                            qcs = [qc for qc in range(NQC)
                                   if kt < (qc + 1) * KT_PER_QC]
                            nxt = []
                            for qc in qcs:
                                sc_psum = sc_tile()
                                nc.tensor.matmul(
                                    sc_psum[:],
                                    kh_sb[fo:fo + D, ft, kt * P:(kt + 1) * P],
                                    qh_sb[fo:fo + D, ft, qc * QC:(qc + 1) * QC],
                                    start=True, stop=True)
                                at = attnp.tile([P, QC], BF16, tag="at")
                                emit_exp(at, sc_psum, kt, qc, None)
                                nxt.append((kt, qc, at))
                            pend.append(nxt)
                            if len(pend) > XTLAG:
                                flush_xt(pend.pop(0))
                        for gen in pend:
                            flush_xt(gen)
                        for qc in range(NQC):
                            normalize(xt_psums[qc], h, qc)

                elif run2:
                    # general-mask path: qc-outer, mask tiles streamed per qc.
                    # (also used as the old_p2 comparison structure for causal)
                    for qc in range(NQC):
                        if causal:
                            mc = None
                            ktm = (qc + 1) * KT_PER_QC
                        else:
                            mc = streams.tile([P, ST, QC], BF16, tag="mc")
                            nc.sync.dma_start(mc[:], maskT3[:, :, qc * QC:(qc + 1) * QC])
                            ktm = ST
                        for h in range(H):
                            ft, fo = h // 2, (h % 2) * D
                            xt_psum = ps_xt.tile([D + 1, QC], F32, tag="xt0")
                            at_tiles = [None] * ktm

                            def emit_sc(kt):
                                sc_psum = sc_tile()
                                nc.tensor.matmul(
                                    sc_psum[:],
                                    kh_sb[fo:fo + D, ft, kt * P:(kt + 1) * P],
                                    qh_sb[fo:fo + D, ft, qc * QC:(qc + 1) * QC],
                                    start=True, stop=True)
                                at = attnp.tile([P, QC], BF16, tag="at")
                                emit_exp(at, sc_psum, kt, qc, mc)
                                at_tiles[kt] = at

                            def emit_xt(kt):
                                nc.tensor.matmul(
                                    xt_psum[:],
                                    vh_sb[:, kt, h, :],
                                    at_tiles[kt][:],
                                    start=(kt == 0), stop=(kt == ktm - 1))

                            PIPE = 2
                            for kt in range(ktm):
                                emit_sc(kt)
                                if kt >= PIPE:
                                    emit_xt(kt - PIPE)
                            for kt in range(max(0, ktm - PIPE), ktm):
                                emit_xt(kt)
                            normalize(xt_psum, h, qc)

                # ---- Phase 3: output projection (partial over local heads) ----
                for jt in range(ET) if run3 else ():
                    if p3_wide > 1:
                        psums = [ps1.tile([P, QC], F32, tag="ps", name=f"po{qc}")
                                 for qc in range(NQC)]
                        for ft in range(FT):
                            for qc in range(NQC):
                                nc.tensor.matmul(
                                    psums[qc][:],
                                    wo_sb[:, ft, jt * P:(jt + 1) * P],
                                    xts_sb[:, ft, qc * QC:(qc + 1) * QC],
                                    start=(ft == 0), stop=(ft == FT - 1))
                        for qc in range(NQC):
                            ot = streams.tile([P, QC], F32, tag="ot")
                            nc.vector.tensor_copy(ot[:], psums[qc][:])
                            nc.sync.dma_start(
                                outT[jt * P:(jt + 1) * P, qc * QC:(qc + 1) * QC],
                                ot[:])
                    else:
                        for qc in range(NQC):
                            psum = ps1.tile([P, QC], F32, tag="ps", name="po")
                            for ft in range(FT):
                                nc.tensor.matmul(
                                    psum[:],
                                    wo_sb[:, ft, jt * P:(jt + 1) * P],
                                    xts_sb[:, ft, qc * QC:(qc + 1) * QC],
                                    start=(ft == 0), stop=(ft == FT - 1))
                            ot = streams.tile([P, QC], F32, tag="ot")
                            nc.vector.tensor_copy(ot[:], psum[:])
                            nc.sync.dma_start(
                                outT[jt * P:(jt + 1) * P, qc * QC:(qc + 1) * QC],
                                ot[:])

            if niter is None:
                body()
            else:
                with tc.For_i(0, niter, 1):
                    body()

    nc.compile()
    return nc


def _host_prep(q, k, v, mask, w_q, w_k, w_v, w_o):
    """Shard + transpose inputs on the host.  Returns (in_maps, causal)."""
    tril = np.tril(np.ones((S, S), dtype=mask.dtype))
    causal = all(np.array_equal(np.asarray(mask[b, 0]), tril) for b in range(B))

    stair = (np.arange(2 * QC)[None, :] >= (np.arange(P)[:, None] + QC))
    stair = stair.astype(NPBF16)

    w_q = np.asarray(w_q, dtype=np.float32)
    w_k = np.asarray(w_k, dtype=np.float32)
    w_v = np.asarray(w_v, dtype=np.float32)
    w_o = np.asarray(w_o, dtype=np.float32)

    in_maps = []
    for core in range(8):
        b, g = divmod(core, 2)
        rows = slice(g * F, (g + 1) * F)
        m = {
            "qT": np.ascontiguousarray(np.asarray(q[b], np.float32).T).astype(NPBF16),
            "kT": np.ascontiguousarray(np.asarray(k[b], np.float32).T).astype(NPBF16),
            "vT": np.ascontiguousarray(np.asarray(v[b], np.float32).T).astype(NPBF16),
            "wqT": np.ascontiguousarray(w_q[rows, :].T).astype(NPBF16),
            "wkT": np.ascontiguousarray(w_k[rows, :].T).astype(NPBF16),
            "wvT": np.ascontiguousarray(w_v[rows, :].T).astype(NPBF16),
            "woT": np.ascontiguousarray(w_o[:, rows].T).astype(NPBF16),
            "stair": stair,
        }
        if not causal:
            m["maskT"] = np.ascontiguousarray(
                np.asarray(mask[b, 0], np.float32).T).astype(NPBF16)
        in_maps.append(m)
    return in_maps, causal


_NC_CACHE: dict = {}


def kernel(q, k, v, mask, w_q, w_k, w_v, w_o):
    in_maps, causal = _host_prep(q, k, v, mask, w_q, w_k, w_v, w_o)
    nc = _NC_CACHE.get(causal)
    if nc is None:
        nc = build_nc(causal)
        _NC_CACHE[causal] = nc
    res = bass_utils.run_bass_kernel_spmd(nc, in_maps, core_ids=list(range(8)))
    out = np.empty((B, S, E), dtype=np.float32)
    for b in range(B):
        out[b] = (res.results[2 * b]["outT"] + res.results[2 * b + 1]["outT"]).T
    return out



# revision 4
# speedup vs baseline: 1.0041x; 1.0041x over previous
"""Multi-head attention block kernel for Trainium2, sharded over 8 NeuronCores.

Sharding: batch (4) x head-group (2 groups of 8 heads) -> 8 cores.
Each core computes, for one batch b and one half of the heads:
  qh/kh/vh projections (columns of w_q/w_k/w_v for its heads),
  causal attention for its 8 heads, and a partial output projection
  (rows of w_o^T for its heads).  Host sums the two partial outputs per
  batch and transposes back.

On-chip layout is feature-major ("transposed"): activations live as
[feature, seq] so every matmul contraction dim is on partitions and no
on-chip transposes are needed.  Host pre-transposes q/k/v and the
weight slices, and post-transposes the output.

Matmuls run in bf16 (fp32 matmul is 4x slower on TRN2); accumulation is
fp32 in PSUM.  Softmax denominators come for free from an extra ones
column appended to each V tile (row 64 of the attn@V accumulator is the
sum of exp scores).
"""

import sys

sys.path.insert(0, "/opt/trn_rl_repo")

import numpy as np
import ml_dtypes

import concourse.bacc as bacc
import concourse.mybir as mybir
import concourse.tile as tile
from concourse import bass_utils

B = 4
S = 2048
E = 1024
HEADS = 16
D = 64
H = 8            # heads per core
F = H * D        # 512 local head features
P = 128
ET = E // P      # 8 e-tiles
FT = F // P      # 4 f-tiles
ST = S // P      # 16 s-tiles
QC = 512         # q-chunk width
NQC = S // QC    # 4 q-chunks
KT_PER_QC = QC // P  # 4 k-tiles per q-chunk

BF16 = mybir.dt.bfloat16
F32 = mybir.dt.float32
NPBF16 = ml_dtypes.bfloat16


def build_nc(causal: bool, niter: int | None = None, phases=(1, 2, 3), no_norm=False, no_exp=False,
             p1_wide=4, p3_wide=4, xtlag=2, sc_bufs=0, ps_bufs=4, at_bufs=12, old_p2=False):
    """Build the per-core Bass program.  If niter is given, wrap the body in a
    For_i timing loop (used by test.py to measure HW time)."""
    nc = bacc.Bacc("TRN2", target_bir_lowering=False, debug=False,
                   enable_asserts=True, num_devices=8)

    qT = nc.dram_tensor("qT", [E, S], BF16, kind="ExternalInput").ap()
    kT = nc.dram_tensor("kT", [E, S], BF16, kind="ExternalInput").ap()
    vT = nc.dram_tensor("vT", [E, S], BF16, kind="ExternalInput").ap()
    wqT = nc.dram_tensor("wqT", [E, F], BF16, kind="ExternalInput").ap()
    wkT = nc.dram_tensor("wkT", [E, F], BF16, kind="ExternalInput").ap()
    wvT = nc.dram_tensor("wvT", [E, F], BF16, kind="ExternalInput").ap()
    woT = nc.dram_tensor("woT", [F, E], BF16, kind="ExternalInput").ap()
    stair = nc.dram_tensor("stair", [P, 2 * QC], BF16, kind="ExternalInput").ap()
    if not causal:
        maskT = nc.dram_tensor("maskT", [S, S], BF16, kind="ExternalInput").ap()
    outT = nc.dram_tensor("outT", [E, S], F32, kind="ExternalOutput").ap()

    qT3 = qT.rearrange("(o p) s -> p o s", p=P)
    kT3 = kT.rearrange("(o p) s -> p o s", p=P)
    vT3 = vT.rearrange("(o p) s -> p o s", p=P)
    if not causal:
        maskT3 = maskT.rearrange("(o p) s -> p o s", p=P)

    with tile.TileContext(nc) as tc:
        import contextlib
        with contextlib.ExitStack() as ctx:
            persist = ctx.enter_context(tc.tile_pool(name="persist", bufs=1))
            streams = ctx.enter_context(tc.tile_pool(name="streams", bufs=6))
            attnp = ctx.enter_context(tc.tile_pool(name="attnp", bufs=at_bufs))
            smalls = ctx.enter_context(tc.tile_pool(name="smalls", bufs=3))
            ps1 = ctx.enter_context(tc.tile_pool(name="ps1", bufs=ps_bufs, space="PSUM"))
            if sc_bufs:
                ps_sc = ctx.enter_context(tc.tile_pool(name="ps_sc", bufs=sc_bufs, space="PSUM"))
            ps_xt = ctx.enter_context(tc.tile_pool(name="ps_xt", bufs=1, space="PSUM"))
            def sc_tile():
                if sc_bufs:
                    return ps_sc.tile([P, QC], F32, tag="sc", name="scp")
                return ps1.tile([P, QC], F32, tag="ps", name="scp")

            # Weights + constants: loaded once, outside the timing loop.
            wq_sb = persist.tile([P, ET, F], BF16, tag="wq")
            wk_sb = persist.tile([P, ET, F], BF16, tag="wk")
            wv_sb = persist.tile([P, ET, F], BF16, tag="wv")
            wo_sb = persist.tile([P, FT, E], BF16, tag="wo")
            stair_sb = persist.tile([P, 2 * QC], BF16, tag="stair")
            nc.sync.dma_start(wq_sb[:], wqT.rearrange("(o p) f -> p o f", p=P))
            nc.sync.dma_start(wk_sb[:], wkT.rearrange("(o p) f -> p o f", p=P))
            nc.sync.dma_start(wv_sb[:], wvT.rearrange("(o p) f -> p o f", p=P))
            nc.sync.dma_start(wo_sb[:], woT.rearrange("(o p) e -> p o e", p=P))
            nc.sync.dma_start(stair_sb[:], stair[:])

            # Persistent activations (bf16): projections and attention outputs.
            qh_sb = persist.tile([P, FT, S], BF16, tag="qh")    # [f, ft, s]
            kh_sb = persist.tile([P, FT, S], BF16, tag="kh")
            vh_sb = persist.tile([P, ST, H, D + 1], BF16, tag="vh")  # ones col at d=64
            xts_sb = persist.tile([P, FT, S], BF16, tag="xts")

            def body():
                run1 = 1 in phases
                run2 = 2 in phases
                run3 = 3 in phases
                if not run1:
                    nc.vector.memset(qh_sb[:, :, 0:1], 0.5)
                    nc.vector.memset(kh_sb[:, :, 0:1], 0.5)
                    nc.vector.memset(vh_sb[:, :, :, 0:1], 0.5)
                if not run2 and run3:
                    nc.vector.memset(xts_sb[:, :, 0:1], 0.5)
                # ---- Phase 1a: q/k projections -> qh/kh (feature-major) ----
                # Weight-stationary: for each (ft, et) weight tile, stream all
                # 4 s-chunks into 4 accumulating PSUMs so LDWEIGHTS happens
                # once per 4 matmuls.
                for src3, w_sb, dst in ((qT3, wq_sb, qh_sb), (kT3, wk_sb, kh_sb)) if run1 else ():
                    xcs = []
                    for sc in range(NQC):
                        xc = streams.tile([P, ET, QC], BF16, tag="xc")
                        nc.sync.dma_start(xc[:], src3[:, :, sc * QC:(sc + 1) * QC])
                        xcs.append(xc)
                    for ft in range(FT):
                        if p1_wide > 1:
                            for g0 in range(0, NQC, p1_wide):
                                gs = list(range(g0, min(NQC, g0 + p1_wide)))
                                psums = [ps1.tile([P, QC], F32, tag="ps", name=f"pp{sc}")
                                         for sc in gs]
                                for et in range(ET):
                                    for i, sc in enumerate(gs):
                                        nc.tensor.matmul(
                                            psums[i][:],
                                            w_sb[:, et, ft * P:(ft + 1) * P],
                                            xcs[sc][:, et, :],
                                            start=(et == 0), stop=(et == ET - 1))
                                for i, sc in enumerate(gs):
                                    nc.vector.tensor_copy(
                                        dst[:, ft, sc * QC:(sc + 1) * QC], psums[i][:])
                        else:
                            for sc in range(NQC):
                                psum = ps1.tile([P, QC], F32, tag="ps", name="pp")
                                for et in range(ET):
                                    nc.tensor.matmul(
                                        psum[:],
                                        w_sb[:, et, ft * P:(ft + 1) * P],
                                        xcs[sc][:, et, :],
                                        start=(et == 0), stop=(et == ET - 1))
                                nc.vector.tensor_copy(
                                    dst[:, ft, sc * QC:(sc + 1) * QC], psum[:])

                # ---- Phase 1b: v projection -> vh (seq-major) + ones column ----
                nc.vector.memset(vh_sb[:, :, :, D:D + 1], 1.0)
                for sc in range(NQC) if run1 else ():
                    xc = streams.tile([P, ET, QC], BF16, tag="xc")
                    nc.sync.dma_start(xc[:], vT3[:, :, sc * QC:(sc + 1) * QC])
                    for si in range(KT_PER_QC):
                        st = sc * KT_PER_QC + si
                        psum = ps1.tile([P, QC], F32, tag="ps")
                        for et in range(ET):
                            nc.tensor.matmul(
                                psum[:],
                                xc[:, et, si * P:(si + 1) * P],
                                wv_sb[:, et, :],
                                start=(et == 0), stop=(et == ET - 1))
                        nc.vector.tensor_copy(
                            vh_sb[:, st, :, 0:D],
                            psum[:].rearrange("p (h d) -> p h d", h=H))

                # ---- Phase 2: attention ----
                # Causal path: kt-outer so the stationary operands (K tile for
                # scores, V tile for attn@V) are each loaded once per (h, kt)
                # and streamed over all valid q-chunks (LDWEIGHTS amortization;
                # weight switches cost ~250ns on PE).  Needs one xt accumulator
                # per q-chunk (4 PSUM banks).
                def normalize(xt_psum, h, qc):
                    ft, fo = h // 2, (h % 2) * D
                    if no_norm:
                        nc.vector.tensor_copy(
                            xts_sb[fo:fo + D, ft, qc * QC:(qc + 1) * QC],
                            xt_psum[0:D, :])
                    else:
                        recip = smalls.tile([1, QC], F32, tag="recip")
                        nc.vector.reciprocal(recip[:], xt_psum[D:D + 1, :])
                        rb = smalls.tile([D, QC], F32, tag="rb")
                        nc.gpsimd.partition_broadcast(rb[:], recip[0:1, :])
                        nc.vector.tensor_mul(
                            xts_sb[fo:fo + D, ft, qc * QC:(qc + 1) * QC],
                            xt_psum[0:D, :], rb[:])

                def emit_exp(at, sc_psum, kt, qc, mc):
                    if no_exp:
                        nc.vector.tensor_copy(at[:], sc_psum[:])
                    else:
                        nc.scalar.activation(at[:], sc_psum[:],
                                             mybir.ActivationFunctionType.Exp,
                                             scale=0.125)
                    if causal:
                        if kt // KT_PER_QC == qc:
                            off = kt * P - qc * QC
                            nc.vector.tensor_mul(
                                at[:], at[:], stair_sb[:, QC - off:2 * QC - off])
                    else:
                        nc.vector.tensor_mul(at[:], at[:], mc[:, kt, :])

                if run2 and causal and not old_p2:
                    for h in range(H):
                        ft, fo = h // 2, (h % 2) * D
                        xt_psums = [ps_xt.tile([D + 1, QC], F32, tag=f"xt{qc}", name=f"xt{qc}")
                                    for qc in range(NQC)]
                        pend = []   # [(kt, qc, at)] generations awaiting attn@V

                        def flush_xt(gen):
                            for kt, qc, at in gen:
                                nc.tensor.matmul(
                                    xt_psums[qc][:],
                                    vh_sb[:, kt, h, :],
                                    at[:],
                                    start=(kt == 0),
                                    stop=(kt == (qc + 1) * KT_PER_QC - 1))

                        XTLAG = xtlag
                        for kt in range(ST):
                            qcs = [qc for qc in range(NQC)
                                   if kt < (qc + 1) * KT_PER_QC]
                            nxt = []
                            for qc in qcs:
                                sc_psum = sc_tile()
                                nc.tensor.matmul(
                                    sc_psum[:],
                                    kh_sb[fo:fo + D, ft, kt * P:(kt + 1) * P],
                                    qh_sb[fo:fo + D, ft, qc * QC:(qc + 1) * QC],
                                    start=True, stop=True)
                                at = attnp.tile([P, QC], BF16, tag="at")
                                emit_exp(at, sc_psum, kt, qc, None)
                                nxt.append((kt, qc, at))
                            pend.append(nxt)
                            if len(pend) > XTLAG:
                                flush_xt(pend.pop(0))
                        for gen in pend:
                            flush_xt(gen)
                        for qc in range(NQC):
                            normalize(xt_psums[qc], h, qc)

                elif run2:
                    # general-mask path: qc-outer, mask tiles streamed per qc.
                    # (also used as the old_p2 comparison structure for causal)
                    for qc in range(NQC):
                        if causal:
                            mc = None
                            ktm = (qc + 1) * KT_PER_QC
                        else:
                            mc = streams.tile([P, ST, QC], BF16, tag="mc")
                            nc.sync.dma_start(mc[:], maskT3[:, :, qc * QC:(qc + 1) * QC])
                            ktm = ST
                        for h in range(H):
                            ft, fo = h // 2, (h % 2) * D
                            xt_psum = ps_xt.tile([D + 1, QC], F32, tag="xt0")
                            at_tiles = [None] * ktm

                            def emit_sc(kt):
                                sc_psum = sc_tile()
                                nc.tensor.matmul(
                                    sc_psum[:],
                                    kh_sb[fo:fo + D, ft, kt * P:(kt + 1) * P],
                                    qh_sb[fo:fo + D, ft, qc * QC:(qc + 1) * QC],
                                    start=True, stop=True)
                                at = attnp.tile([P, QC], BF16, tag="at")
                                emit_exp(at, sc_psum, kt, qc, mc)
                                at_tiles[kt] = at

                            def emit_xt(kt):
                                nc.tensor.matmul(
                                    xt_psum[:],
                                    vh_sb[:, kt, h, :],
                                    at_tiles[kt][:],
                                    start=(kt == 0), stop=(kt == ktm - 1))

                            PIPE = 2
                            for kt in range(ktm):
                                emit_sc(kt)
                                if kt >= PIPE:
                                    emit_xt(kt - PIPE)
                            for kt in range(max(0, ktm - PIPE), ktm):
                                emit_xt(kt)
                            normalize(xt_psum, h, qc)

                # ---- Phase 3: output projection (partial over local heads) ----
                for jt in range(ET) if run3 else ():
                    if p3_wide > 1:
                        psums = [ps1.tile([P, QC], F32, tag="ps", name=f"po{qc}")
                                 for qc in range(NQC)]
                        for ft in range(FT):
                            for qc in range(NQC):
                                nc.tensor.matmul(
                                    psums[qc][:],
                                    wo_sb[:, ft, jt * P:(jt + 1) * P],
                                    xts_sb[:, ft, qc * QC:(qc + 1) * QC],
                                    start=(ft == 0), stop=(ft == FT - 1))
                        for qc in range(NQC):
                            ot = streams.tile([P, QC], F32, tag="ot")
                            nc.vector.tensor_copy(ot[:], psums[qc][:])
                            nc.sync.dma_start(
                                outT[jt * P:(jt + 1) * P, qc * QC:(qc + 1) * QC],
                                ot[:])
                    else:
                        for qc in range(NQC):
                            psum = ps1.tile([P, QC], F32, tag="ps", name="po")
                            for ft in range(FT):
                                nc.tensor.matmul(
                                    psum[:],
                                    wo_sb[:, ft, jt * P:(jt + 1) * P],
                                    xts_sb[:, ft, qc * QC:(qc + 1) * QC],
                                    start=(ft == 0), stop=(ft == FT - 1))
                            ot = streams.tile([P, QC], F32, tag="ot")
                            nc.vector.tensor_copy(ot[:], psum[:])
                            nc.sync.dma_start(
                                outT[jt * P:(jt + 1) * P, qc * QC:(qc + 1) * QC],
                                ot[:])

            if niter is None:
                body()
            else:
                with tc.For_i(0, niter, 1):
                    body()

    nc.compile()
    return nc


def _plan_groups(qc):
    """kt-tile groups for one (h, qc) block: non-diag groups of <=3 (no mask),
    then the 4 diagonal tiles as [3, 1] with fixed mask-table slices."""
    nd = 4 * qc
    groups = []
    k0 = 0
    while k0 < nd:
        n = min(3, nd - k0)
        groups.append((k0, n, None))
        k0 += n
    groups.append((nd, 3, 0))            # diag tiles jd=0..2 -> dmask cols [0, 1536)
    groups.append((nd + 3, 1, 3 * QC))   # diag tile jd=3 -> dmask cols [1536, 2048)
    return groups


def build_nc2(niter=None, lag=1, at3_bufs=3, qk_copy="scalar"):
    """Causal-only v2: qc-major waves, batched exp over 3-bank PSUM groups,
    proj/out-proj units interleaved into the attention stream as PE filler."""
    nc = bacc.Bacc("TRN2", target_bir_lowering=False, debug=False,
                   enable_asserts=True, num_devices=8)

    qT = nc.dram_tensor("qT", [E, S], BF16, kind="ExternalInput").ap()
    kT = nc.dram_tensor("kT", [E, S], BF16, kind="ExternalInput").ap()
    vT = nc.dram_tensor("vT", [E, S], BF16, kind="ExternalInput").ap()
    wqT = nc.dram_tensor("wqT", [E, F], BF16, kind="ExternalInput").ap()
    wkT = nc.dram_tensor("wkT", [E, F], BF16, kind="ExternalInput").ap()
    wvT = nc.dram_tensor("wvT", [E, F], BF16, kind="ExternalInput").ap()
    woT = nc.dram_tensor("woT", [F, E], BF16, kind="ExternalInput").ap()
    dmask = nc.dram_tensor("dmask", [P, 4 * QC], BF16, kind="ExternalInput").ap()
    outT = nc.dram_tensor("outT", [E, S], F32, kind="ExternalOutput").ap()

    qT3 = qT.rearrange("(o p) s -> p o s", p=P)
    kT3 = kT.rearrange("(o p) s -> p o s", p=P)
    vT3 = vT.rearrange("(o p) s -> p o s", p=P)

    with tile.TileContext(nc) as tc:
        import contextlib
        with contextlib.ExitStack() as ctx:
            persist = ctx.enter_context(tc.tile_pool(name="persist", bufs=1))
            streams = ctx.enter_context(tc.tile_pool(name="streams", bufs=2))
            otp = ctx.enter_context(tc.tile_pool(name="otp", bufs=3))
            attnp = ctx.enter_context(tc.tile_pool(name="attnp", bufs=at3_bufs))
            smalls = ctx.enter_context(tc.tile_pool(name="smalls", bufs=3))
            ps_sc = ctx.enter_context(tc.tile_pool(name="ps_sc", bufs=2, space="PSUM"))
            ps_xt = ctx.enter_context(tc.tile_pool(name="ps_xt", bufs=1, space="PSUM"))
            ps_pp = ctx.enter_context(tc.tile_pool(name="ps_pp", bufs=1, space="PSUM"))

            wq_sb = persist.tile([P, ET, F], BF16, tag="wq")
            wk_sb = persist.tile([P, ET, F], BF16, tag="wk")
            wv_sb = persist.tile([P, ET, F], BF16, tag="wv")
            wo_sb = persist.tile([P, FT, E], BF16, tag="wo")
            dmask_sb = persist.tile([P, 4 * QC], BF16, tag="dmask")
            nc.sync.dma_start(wq_sb[:], wqT.rearrange("(o p) f -> p o f", p=P))
            nc.sync.dma_start(wk_sb[:], wkT.rearrange("(o p) f -> p o f", p=P))
            nc.sync.dma_start(wv_sb[:], wvT.rearrange("(o p) f -> p o f", p=P))
            nc.sync.dma_start(wo_sb[:], woT.rearrange("(o p) e -> p o e", p=P))
            nc.sync.dma_start(dmask_sb[:], dmask[:])

            qh_sb = persist.tile([P, FT, S], BF16, tag="qh")
            kh_sb = persist.tile([P, FT, S], BF16, tag="kh")
            vh_sb = persist.tile([P, ST, H, D + 1], BF16, tag="vh")
            xts_sb = persist.tile([P, FT, S], BF16, tag="xts")

            def body():
                nc.vector.memset(vh_sb[:, :, :, D:D + 1], 1.0)

                def dma_wave(sc):
                    w = {}
                    for tag, src3 in (("xq", qT3), ("xk", kT3), ("xv", vT3)):
                        t = streams.tile([P, ET, QC], BF16, tag=tag)
                        nc.sync.dma_start(t[:], src3[:, :, sc * QC:(sc + 1) * QC])
                        w[tag] = t
                    return w

                def qk_unit(wave, which, ft, sc):
                    w_sb, dst, xc = ((wq_sb, qh_sb, wave["xq"]) if which == "q"
                                     else (wk_sb, kh_sb, wave["xk"]))
                    psum = ps_pp.tile([P, QC], F32, tag="pp")
                    for et in range(ET):
                        nc.tensor.matmul(psum[:], w_sb[:, et, ft * P:(ft + 1) * P],
                                         xc[:, et, :],
                                         start=(et == 0), stop=(et == ET - 1))
                    if qk_copy == "scalar":
                        nc.scalar.copy(dst[:, ft, sc * QC:(sc + 1) * QC], psum[:])
                    else:
                        nc.vector.tensor_copy(dst[:, ft, sc * QC:(sc + 1) * QC], psum[:])

                def v_unit(wave, st):
                    si = st % KT_PER_QC
                    psum = ps_pp.tile([P, QC], F32, tag="pp")
                    for et in range(ET):
                        nc.tensor.matmul(psum[:], wave["xv"][:, et, si * P:(si + 1) * P],
                                         wv_sb[:, et, :],
                                         start=(et == 0), stop=(et == ET - 1))
                    nc.vector.tensor_copy(
                        vh_sb[:, st, :, 0:D],
                        psum[:].rearrange("p (h d) -> p h d", h=H))

                def p3_unit(jt, qc):
                    psum = ps_pp.tile([P, QC], F32, tag="pp")
                    for ft in range(FT):
                        nc.tensor.matmul(psum[:], wo_sb[:, ft, jt * P:(jt + 1) * P],
                                         xts_sb[:, ft, qc * QC:(qc + 1) * QC],
                                         start=(ft == 0), stop=(ft == FT - 1))
                    ot = otp.tile([P, QC], F32, tag="ot")
                    nc.vector.tensor_copy(ot[:], psum[:])
                    nc.sync.dma_start(
                        outT[jt * P:(jt + 1) * P, qc * QC:(qc + 1) * QC], ot[:])

                def head_block(h, qc):
                    ft, fo = h // 2, (h % 2) * D
                    xt_t = ps_xt.tile([D + 1, QC], F32, tag="xt")
                    pend = []

                    def flush_one():
                        k0, n, at_t = pend.pop(0)
                        for j in range(n):
                            kt = k0 + j
                            nc.tensor.matmul(xt_t[:], vh_sb[:, kt, h, :],
                                             at_t[:, j * QC:(j + 1) * QC],
                                             start=(kt == 0), stop=(kt == 4 * qc + 3))

                    for (k0, n, mcol) in _plan_groups(qc):
                        sc_t = ps_sc.tile([P, 3 * QC], F32, tag="sc")
                        for j in range(n):
                            kt = k0 + j
                            nc.tensor.matmul(
                                sc_t[:, j * QC:(j + 1) * QC],
                                kh_sb[fo:fo + D, ft, kt * P:(kt + 1) * P],
                                qh_sb[fo:fo + D, ft, qc * QC:(qc + 1) * QC],
                                start=True, stop=True)
                        w = n * QC
                        at_t = attnp.tile([P, 3 * QC] if n > 1 else [P, QC], BF16,
                                          tag=("at3" if n > 1 else "at1"))
                        nc.scalar.activation(at_t[:, 0:w], sc_t[:, 0:w],
                                             mybir.ActivationFunctionType.Exp,
                                             scale=0.125)
                        if mcol is not None:
                            nc.vector.tensor_mul(at_t[:, 0:w], at_t[:, 0:w],
                                                 dmask_sb[:, mcol:mcol + w])
                        pend.append((k0, n, at_t))
                        if len(pend) > lag:
                            flush_one()
                    while pend:
                        flush_one()
                    # normalize
                    recip = smalls.tile([1, QC], F32, tag="recip")
                    nc.vector.reciprocal(recip[:], xt_t[D:D + 1, :])
                    rb = smalls.tile([D, QC], F32, tag="rb")
                    nc.gpsimd.partition_broadcast(rb[:], recip[0:1, :])
                    nc.vector.tensor_mul(
                        xts_sb[fo:fo + D, ft, qc * QC:(qc + 1) * QC],
                        xt_t[0:D, :], rb[:])

                # ---- lead-in ----
                cur = dma_wave(0)
                qk_unit(cur, "q", 0, 0)
                qk_unit(cur, "k", 0, 0)
                for st in range(KT_PER_QC):
                    v_unit(cur, st)

                # ---- waves ----
                for qc in range(NQC):
                    nxt = dma_wave(qc + 1) if qc + 1 < NQC else None
                    proj_fill = ([(s, f) for f in range(FT) for s in ("q", "k")]
                                 if nxt else [])
                    v_fill = ([4 * (qc + 1) + i for i in range(KT_PER_QC)]
                              if nxt else [])
                    p3_fill = [(jt, qc - 1) for jt in range(ET)] if qc >= 1 else []
                    for h in range(H):
                        if qc == 0 and h >= 2 and h % 2 == 0:
                            qk_unit(cur, "q", h // 2, 0)
                            qk_unit(cur, "k", h // 2, 0)
                        head_block(h, qc)
                        if nxt:
                            s, f = proj_fill[h]
                            qk_unit(nxt, s, f, qc + 1)
                            if h % 2 == 1:
                                v_unit(nxt, v_fill[h // 2])
                        if p3_fill:
                            jt, qcp = p3_fill[h]
                            p3_unit(jt, qcp)
                    cur = nxt

                # ---- tail ----
                for jt in range(ET):
                    p3_unit(jt, NQC - 1)

            if niter is None:
                body()
            else:
                with tc.For_i(0, niter, 1):
                    body()

    nc.compile()
    return nc


def _host_prep(q, k, v, mask, w_q, w_k, w_v, w_o):
    """Shard + transpose inputs on the host.  Returns (in_maps, causal)."""
    tril = np.tril(np.ones((S, S), dtype=mask.dtype))
    causal = all(np.array_equal(np.asarray(mask[b, 0]), tril) for b in range(B))

    stair = (np.arange(2 * QC)[None, :] >= (np.arange(P)[:, None] + QC))
    stair = stair.astype(NPBF16)

    # v2 diag mask: 4 concatenated [P, QC] tiles, tile jd valid iff q >= p + 128*jd
    dmask = np.concatenate(
        [(np.arange(QC)[None, :] >= (np.arange(P)[:, None] + P * jd))
         for jd in range(4)], axis=1).astype(NPBF16)

    w_q = np.asarray(w_q, dtype=np.float32)
    w_k = np.asarray(w_k, dtype=np.float32)
    w_v = np.asarray(w_v, dtype=np.float32)
    w_o = np.asarray(w_o, dtype=np.float32)

    in_maps = []
    for core in range(8):
        b, g = divmod(core, 2)
        rows = slice(g * F, (g + 1) * F)
        m = {
            "qT": np.ascontiguousarray(np.asarray(q[b], np.float32).T).astype(NPBF16),
            "kT": np.ascontiguousarray(np.asarray(k[b], np.float32).T).astype(NPBF16),
            "vT": np.ascontiguousarray(np.asarray(v[b], np.float32).T).astype(NPBF16),
            "wqT": np.ascontiguousarray(w_q[rows, :].T).astype(NPBF16),
            "wkT": np.ascontiguousarray(w_k[rows, :].T).astype(NPBF16),
            "wvT": np.ascontiguousarray(w_v[rows, :].T).astype(NPBF16),
            "woT": np.ascontiguousarray(w_o[:, rows].T).astype(NPBF16),
            "stair": stair,
            "dmask": dmask,
        }
        if not causal:
            m["maskT"] = np.ascontiguousarray(
                np.asarray(mask[b, 0], np.float32).T).astype(NPBF16)
        in_maps.append(m)
    return in_maps, causal


_NC_CACHE: dict = {}


def kernel(q, k, v, mask, w_q, w_k, w_v, w_o):
    in_maps, causal = _host_prep(q, k, v, mask, w_q, w_k, w_v, w_o)
    nc = _NC_CACHE.get(causal)
    if nc is None:
        nc = build_nc2() if causal else build_nc(causal)
        _NC_CACHE[causal] = nc
    res = bass_utils.run_bass_kernel_spmd(nc, in_maps, core_ids=list(range(8)))
    out = np.empty((B, S, E), dtype=np.float32)
    for b in range(B):
        out[b] = (res.results[2 * b]["outT"] + res.results[2 * b + 1]["outT"]).T
    return out



# revision 16
# speedup vs baseline: 1.1215x; 1.1170x over previous
"""Multi-head attention block kernel for Trainium2, sharded over 8 NeuronCores.

Sharding: batch (4) x head-group (2 groups of 8 heads) -> 8 cores.
Each core computes, for one batch b and one half of the heads:
  qh/kh/vh projections (columns of w_q/w_k/w_v for its heads),
  causal attention for its 8 heads, and a partial output projection
  (rows of w_o^T for its heads).  Host sums the two partial outputs per
  batch and transposes back.

On-chip layout is feature-major ("transposed"): activations live as
[feature, seq] so every matmul contraction dim is on partitions and no
on-chip transposes are needed.  Host pre-transposes q/k/v and the
weight slices, and post-transposes the output.

Matmuls run in bf16 (fp32 matmul is 4x slower on TRN2); accumulation is
fp32 in PSUM.  Softmax denominators come for free from an extra ones
column appended to each V tile (row 64 of the attn@V accumulator is the
sum of exp scores).
"""

import sys

sys.path.insert(0, "/opt/trn_rl_repo")

import numpy as np
import ml_dtypes

import concourse.bacc as bacc
import concourse.mybir as mybir
import concourse.tile as tile
from concourse import bass_utils

B = 4
S = 2048
E = 1024
HEADS = 16
D = 64
H = 8            # heads per core
F = H * D        # 512 local head features
P = 128
ET = E // P      # 8 e-tiles
FT = F // P      # 4 f-tiles
ST = S // P      # 16 s-tiles
QC = 512         # q-chunk width
NQC = S // QC    # 4 q-chunks
KT_PER_QC = QC // P  # 4 k-tiles per q-chunk

BF16 = mybir.dt.bfloat16
F32 = mybir.dt.float32
NPBF16 = ml_dtypes.bfloat16


def build_nc(causal: bool, niter: int | None = None, phases=(1, 2, 3), no_norm=False, no_exp=False,
             p1_wide=4, p3_wide=4, xtlag=2, sc_bufs=0, ps_bufs=4, at_bufs=12, old_p2=False):
    """Build the per-core Bass program.  If niter is given, wrap the body in a
    For_i timing loop (used by test.py to measure HW time)."""
    nc = bacc.Bacc("TRN2", target_bir_lowering=False, debug=False,
                   enable_asserts=True, num_devices=8)

    qT = nc.dram_tensor("qT", [E, S], BF16, kind="ExternalInput").ap()
    kT = nc.dram_tensor("kT", [E, S], BF16, kind="ExternalInput").ap()
    vT = nc.dram_tensor("vT", [E, S], BF16, kind="ExternalInput").ap()
    wqT = nc.dram_tensor("wqT", [E, F], BF16, kind="ExternalInput").ap()
    wkT = nc.dram_tensor("wkT", [E, F], BF16, kind="ExternalInput").ap()
    wvT = nc.dram_tensor("wvT", [E, F], BF16, kind="ExternalInput").ap()
    woT = nc.dram_tensor("woT", [F, E], BF16, kind="ExternalInput").ap()
    stair = nc.dram_tensor("stair", [P, 2 * QC], BF16, kind="ExternalInput").ap()
    if not causal:
        maskT = nc.dram_tensor("maskT", [S, S], BF16, kind="ExternalInput").ap()
    outT = nc.dram_tensor("outT", [E, S], F32, kind="ExternalOutput").ap()

    qT3 = qT.rearrange("(o p) s -> p o s", p=P)
    kT3 = kT.rearrange("(o p) s -> p o s", p=P)
    vT3 = vT.rearrange("(o p) s -> p o s", p=P)
    if not causal:
        maskT3 = maskT.rearrange("(o p) s -> p o s", p=P)

    with tile.TileContext(nc) as tc:
        import contextlib
        with contextlib.ExitStack() as ctx:
            persist = ctx.enter_context(tc.tile_pool(name="persist", bufs=1))
            streams = ctx.enter_context(tc.tile_pool(name="streams", bufs=6))
            attnp = ctx.enter_context(tc.tile_pool(name="attnp", bufs=at_bufs))
            smalls = ctx.enter_context(tc.tile_pool(name="smalls", bufs=3))
            ps1 = ctx.enter_context(tc.tile_pool(name="ps1", bufs=ps_bufs, space="PSUM"))
            if sc_bufs:
                ps_sc = ctx.enter_context(tc.tile_pool(name="ps_sc", bufs=sc_bufs, space="PSUM"))
            ps_xt = ctx.enter_context(tc.tile_pool(name="ps_xt", bufs=1, space="PSUM"))
            def sc_tile():
                if sc_bufs:
                    return ps_sc.tile([P, QC], F32, tag="sc", name="scp")
                return ps1.tile([P, QC], F32, tag="ps", name="scp")

            # Weights + constants: loaded once, outside the timing loop.
            wq_sb = persist.tile([P, ET, F], BF16, tag="wq")
            wk_sb = persist.tile([P, ET, F], BF16, tag="wk")
            wv_sb = persist.tile([P, ET, F], BF16, tag="wv")
            wo_sb = persist.tile([P, FT, E], BF16, tag="wo")
            stair_sb = persist.tile([P, 2 * QC], BF16, tag="stair")
            nc.sync.dma_start(wq_sb[:], wqT.rearrange("(o p) f -> p o f", p=P))
            nc.sync.dma_start(wk_sb[:], wkT.rearrange("(o p) f -> p o f", p=P))
            nc.sync.dma_start(wv_sb[:], wvT.rearrange("(o p) f -> p o f", p=P))
            nc.sync.dma_start(wo_sb[:], woT.rearrange("(o p) e -> p o e", p=P))
            nc.sync.dma_start(stair_sb[:], stair[:])

            # Persistent activations (bf16): projections and attention outputs.
            qh_sb = persist.tile([P, FT, S], BF16, tag="qh")    # [f, ft, s]
            kh_sb = persist.tile([P, FT, S], BF16, tag="kh")
            vh_sb = persist.tile([P, ST, H, D + 1], BF16, tag="vh")  # ones col at d=64
            xts_sb = persist.tile([P, FT, S], BF16, tag="xts")

            def body():
                run1 = 1 in phases
                run2 = 2 in phases
                run3 = 3 in phases
                if not run1:
                    nc.vector.memset(qh_sb[:, :, 0:1], 0.5)
                    nc.vector.memset(kh_sb[:, :, 0:1], 0.5)
                    nc.vector.memset(vh_sb[:, :, :, 0:1], 0.5)
                if not run2 and run3:
                    nc.vector.memset(xts_sb[:, :, 0:1], 0.5)
                # ---- Phase 1a: q/k projections -> qh/kh (feature-major) ----
                # Weight-stationary: for each (ft, et) weight tile, stream all
                # 4 s-chunks into 4 accumulating PSUMs so LDWEIGHTS happens
                # once per 4 matmuls.
                for src3, w_sb, dst in ((qT3, wq_sb, qh_sb), (kT3, wk_sb, kh_sb)) if run1 else ():
                    xcs = []
                    for sc in range(NQC):
                        xc = streams.tile([P, ET, QC], BF16, tag="xc")
                        nc.sync.dma_start(xc[:], src3[:, :, sc * QC:(sc + 1) * QC])
                        xcs.append(xc)
                    for ft in range(FT):
                        if p1_wide > 1:
                            for g0 in range(0, NQC, p1_wide):
                                gs = list(range(g0, min(NQC, g0 + p1_wide)))
                                psums = [ps1.tile([P, QC], F32, tag="ps", name=f"pp{sc}")
                                         for sc in gs]
                                for et in range(ET):
                                    for i, sc in enumerate(gs):
                                        nc.tensor.matmul(
                                            psums[i][:],
                                            w_sb[:, et, ft * P:(ft + 1) * P],
                                            xcs[sc][:, et, :],
                                            start=(et == 0), stop=(et == ET - 1))
                                for i, sc in enumerate(gs):
                                    nc.vector.tensor_copy(
                                        dst[:, ft, sc * QC:(sc + 1) * QC], psums[i][:])
                        else:
                            for sc in range(NQC):
                                psum = ps1.tile([P, QC], F32, tag="ps", name="pp")
                                for et in range(ET):
                                    nc.tensor.matmul(
                                        psum[:],
                                        w_sb[:, et, ft * P:(ft + 1) * P],
                                        xcs[sc][:, et, :],
                                        start=(et == 0), stop=(et == ET - 1))
                                nc.vector.tensor_copy(
                                    dst[:, ft, sc * QC:(sc + 1) * QC], psum[:])

                # ---- Phase 1b: v projection -> vh (seq-major) + ones column ----
                nc.vector.memset(vh_sb[:, :, :, D:D + 1], 1.0)
                for sc in range(NQC) if run1 else ():
                    xc = streams.tile([P, ET, QC], BF16, tag="xc")
                    nc.sync.dma_start(xc[:], vT3[:, :, sc * QC:(sc + 1) * QC])
                    for si in range(KT_PER_QC):
                        st = sc * KT_PER_QC + si
                        psum = ps1.tile([P, QC], F32, tag="ps")
                        for et in range(ET):
                            nc.tensor.matmul(
                                psum[:],
                                xc[:, et, si * P:(si + 1) * P],
                                wv_sb[:, et, :],
                                start=(et == 0), stop=(et == ET - 1))
                        nc.vector.tensor_copy(
                            vh_sb[:, st, :, 0:D],
                            psum[:].rearrange("p (h d) -> p h d", h=H))

                # ---- Phase 2: attention ----
                # Causal path: kt-outer so the stationary operands (K tile for
                # scores, V tile for attn@V) are each loaded once per (h, kt)
                # and streamed over all valid q-chunks (LDWEIGHTS amortization;
                # weight switches cost ~250ns on PE).  Needs one xt accumulator
                # per q-chunk (4 PSUM banks).
                def normalize(xt_psum, h, qc):
                    ft, fo = h // 2, (h % 2) * D
                    if no_norm:
                        nc.vector.tensor_copy(
                            xts_sb[fo:fo + D, ft, qc * QC:(qc + 1) * QC],
                            xt_psum[0:D, :])
                    else:
                        recip = smalls.tile([1, QC], F32, tag="recip")
                        nc.vector.reciprocal(recip[:], xt_psum[D:D + 1, :])
                        rb = smalls.tile([D, QC], F32, tag="rb")
                        nc.gpsimd.partition_broadcast(rb[:], recip[0:1, :])
                        nc.vector.tensor_mul(
                            xts_sb[fo:fo + D, ft, qc * QC:(qc + 1) * QC],
                            xt_psum[0:D, :], rb[:])

                def emit_exp(at, sc_psum, kt, qc, mc):
                    if no_exp:
                        nc.vector.tensor_copy(at[:], sc_psum[:])
                    else:
                        nc.scalar.activation(at[:], sc_psum[:],
                                             mybir.ActivationFunctionType.Exp,
                                             scale=0.125)
                    if causal:
                        if kt // KT_PER_QC == qc:
                            off = kt * P - qc * QC
                            nc.vector.tensor_mul(
                                at[:], at[:], stair_sb[:, QC - off:2 * QC - off])
                    else:
                        nc.vector.tensor_mul(at[:], at[:], mc[:, kt, :])

                if run2 and causal and not old_p2:
                    for h in range(H):
                        ft, fo = h // 2, (h % 2) * D
                        xt_psums = [ps_xt.tile([D + 1, QC], F32, tag=f"xt{qc}", name=f"xt{qc}")
                                    for qc in range(NQC)]
                        pend = []   # [(kt, qc, at)] generations awaiting attn@V

                        def flush_xt(gen):
                            for kt, qc, at in gen:
                                nc.tensor.matmul(
                                    xt_psums[qc][:],
                                    vh_sb[:, kt, h, :],
                                    at[:],
                                    start=(kt == 0),
                                    stop=(kt == (qc + 1) * KT_PER_QC - 1))

                        XTLAG = xtlag
                        for kt in range(ST):
                            qcs = [qc for qc in range(NQC)
                                   if kt < (qc + 1) * KT_PER_QC]
                            nxt = []
                            for qc in qcs:
                                sc_psum = sc_tile()
                                nc.tensor.matmul(
                                    sc_psum[:],
                                    kh_sb[fo:fo + D, ft, kt * P:(kt + 1) * P],
                                    qh_sb[fo:fo + D, ft, qc * QC:(qc + 1) * QC],
                                    start=True, stop=True)
                                at = attnp.tile([P, QC], BF16, tag="at")
                                emit_exp(at, sc_psum, kt, qc, None)
                                nxt.append((kt, qc, at))
                            pend.append(nxt)
                            if len(pend) > XTLAG:
                                flush_xt(pend.pop(0))
                        for gen in pend:
                            flush_xt(gen)
                        for qc in range(NQC):
                            normalize(xt_psums[qc], h, qc)

                elif run2:
                    # general-mask path: qc-outer, mask tiles streamed per qc.
                    # (also used as the old_p2 comparison structure for causal)
                    for qc in range(NQC):
                        if causal:
                            mc = None
                            ktm = (qc + 1) * KT_PER_QC
                        else:
                            mc = streams.tile([P, ST, QC], BF16, tag="mc")
                            nc.sync.dma_start(mc[:], maskT3[:, :, qc * QC:(qc + 1) * QC])
                            ktm = ST
                        for h in range(H):
                            ft, fo = h // 2, (h % 2) * D
                            xt_psum = ps_xt.tile([D + 1, QC], F32, tag="xt0")
                            at_tiles = [None] * ktm

                            def emit_sc(kt):
                                sc_psum = sc_tile()
                                nc.tensor.matmul(
                                    sc_psum[:],
                                    kh_sb[fo:fo + D, ft, kt * P:(kt + 1) * P],
                                    qh_sb[fo:fo + D, ft, qc * QC:(qc + 1) * QC],
                                    start=True, stop=True)
                                at = attnp.tile([P, QC], BF16, tag="at")
                                emit_exp(at, sc_psum, kt, qc, mc)
                                at_tiles[kt] = at

                            def emit_xt(kt):
                                nc.tensor.matmul(
                                    xt_psum[:],
                                    vh_sb[:, kt, h, :],
                                    at_tiles[kt][:],
                                    start=(kt == 0), stop=(kt == ktm - 1))

                            PIPE = 2
                            for kt in range(ktm):
                                emit_sc(kt)
                                if kt >= PIPE:
                                    emit_xt(kt - PIPE)
                            for kt in range(max(0, ktm - PIPE), ktm):
                                emit_xt(kt)
                            normalize(xt_psum, h, qc)

                # ---- Phase 3: output projection (partial over local heads) ----
                for jt in range(ET) if run3 else ():
                    if p3_wide > 1:
                        psums = [ps1.tile([P, QC], F32, tag="ps", name=f"po{qc}")
                                 for qc in range(NQC)]
                        for ft in range(FT):
                            for qc in range(NQC):
                                nc.tensor.matmul(
                                    psums[qc][:],
                                    wo_sb[:, ft, jt * P:(jt + 1) * P],
                                    xts_sb[:, ft, qc * QC:(qc + 1) * QC],
                                    start=(ft == 0), stop=(ft == FT - 1))
                        for qc in range(NQC):
                            ot = streams.tile([P, QC], F32, tag="ot")
                            nc.vector.tensor_copy(ot[:], psums[qc][:])
                            nc.sync.dma_start(
                                outT[jt * P:(jt + 1) * P, qc * QC:(qc + 1) * QC],
                                ot[:])
                    else:
                        for qc in range(NQC):
                            psum = ps1.tile([P, QC], F32, tag="ps", name="po")
                            for ft in range(FT):
                                nc.tensor.matmul(
                                    psum[:],
                                    wo_sb[:, ft, jt * P:(jt + 1) * P],
                                    xts_sb[:, ft, qc * QC:(qc + 1) * QC],
                                    start=(ft == 0), stop=(ft == FT - 1))
                            ot = streams.tile([P, QC], F32, tag="ot")
                            nc.vector.tensor_copy(ot[:], psum[:])
                            nc.sync.dma_start(
                                outT[jt * P:(jt + 1) * P, qc * QC:(qc + 1) * QC],
                                ot[:])

            if niter is None:
                body()
            else:
                with tc.For_i(0, niter, 1):
                    body()

    nc.compile()
    return nc


def _plan_groups(qc):
    """kt-tile groups for one (h, qc) block: non-diag groups of <=3 (no mask),
    then the 4 diagonal tiles as [3, 1] with fixed mask-table slices."""
    nd = 4 * qc
    groups = []
    k0 = 0
    while k0 < nd:
        n = min(3, nd - k0)
        groups.append((k0, n, None))
        k0 += n
    groups.append((nd, 3, 0))            # diag tiles jd=0..2 -> dmask cols [0, 1536)
    groups.append((nd + 3, 1, 3 * QC))   # diag tile jd=3 -> dmask cols [1536, 2048)
    return groups


def build_nc2(niter=None, lag=1, at3_bufs=3, qk_copy="scalar", unroll=1):
    """Causal-only v2: qc-major waves, batched exp over 3-bank PSUM groups,
    proj/out-proj units interleaved into the attention stream as PE filler."""
    nc = bacc.Bacc("TRN2", target_bir_lowering=False, debug=False,
                   enable_asserts=True, num_devices=8)

    qT = nc.dram_tensor("qT", [E, S], BF16, kind="ExternalInput").ap()
    kT = nc.dram_tensor("kT", [E, S], BF16, kind="ExternalInput").ap()
    vT = nc.dram_tensor("vT", [E, S], BF16, kind="ExternalInput").ap()
    wqT = nc.dram_tensor("wqT", [E, F], BF16, kind="ExternalInput").ap()
    wkT = nc.dram_tensor("wkT", [E, F], BF16, kind="ExternalInput").ap()
    wvT = nc.dram_tensor("wvT", [E, F], BF16, kind="ExternalInput").ap()
    woT = nc.dram_tensor("woT", [F, E], BF16, kind="ExternalInput").ap()
    dmask = nc.dram_tensor("dmask", [P, 4 * QC], BF16, kind="ExternalInput").ap()
    outT = nc.dram_tensor("outT", [E, S], F32, kind="ExternalOutput").ap()

    qT3 = qT.rearrange("(o p) s -> p o s", p=P)
    kT3 = kT.rearrange("(o p) s -> p o s", p=P)
    vT3 = vT.rearrange("(o p) s -> p o s", p=P)

    with tile.TileContext(nc) as tc:
        import contextlib
        with contextlib.ExitStack() as ctx:
            persist = ctx.enter_context(tc.tile_pool(name="persist", bufs=1))
            streams = ctx.enter_context(tc.tile_pool(name="streams", bufs=2))
            otp = ctx.enter_context(tc.tile_pool(name="otp", bufs=3))
            attnp = ctx.enter_context(tc.tile_pool(name="attnp", bufs=at3_bufs))
            smalls = ctx.enter_context(tc.tile_pool(name="smalls", bufs=3))
            ps_sc = ctx.enter_context(tc.tile_pool(name="ps_sc", bufs=2, space="PSUM"))
            ps_xt = ctx.enter_context(tc.tile_pool(name="ps_xt", bufs=1, space="PSUM"))
            ps_pp = ctx.enter_context(tc.tile_pool(name="ps_pp", bufs=1, space="PSUM"))

            wq_sb = persist.tile([P, ET, F], BF16, tag="wq")
            wk_sb = persist.tile([P, ET, F], BF16, tag="wk")
            wv_sb = persist.tile([P, ET, F], BF16, tag="wv")
            wo_sb = persist.tile([P, FT, E], BF16, tag="wo")
            dmask_sb = persist.tile([P, 4 * QC], BF16, tag="dmask")
            nc.sync.dma_start(wq_sb[:], wqT.rearrange("(o p) f -> p o f", p=P))
            nc.sync.dma_start(wk_sb[:], wkT.rearrange("(o p) f -> p o f", p=P))
            nc.sync.dma_start(wv_sb[:], wvT.rearrange("(o p) f -> p o f", p=P))
            nc.sync.dma_start(wo_sb[:], woT.rearrange("(o p) e -> p o e", p=P))
            nc.sync.dma_start(dmask_sb[:], dmask[:])

            qh_sb = persist.tile([P, FT, S], BF16, tag="qh")
            kh_sb = persist.tile([P, FT, S], BF16, tag="kh")
            vh_sb = persist.tile([P, ST, H, D + 1], BF16, tag="vh")
            xts_sb = persist.tile([P, FT, S], BF16, tag="xts")

            def body():
                nc.vector.memset(vh_sb[:, :, :, D:D + 1], 1.0)

                def dma_wave(sc):
                    w = {}
                    for tag, src3 in (("xq", qT3), ("xk", kT3), ("xv", vT3)):
                        t = streams.tile([P, ET, QC], BF16, tag=tag)
                        nc.sync.dma_start(t[:], src3[:, :, sc * QC:(sc + 1) * QC])
                        w[tag] = t
                    return w

                def qk_unit(wave, which, ft, sc):
                    w_sb, dst, xc = ((wq_sb, qh_sb, wave["xq"]) if which == "q"
                                     else (wk_sb, kh_sb, wave["xk"]))
                    psum = ps_pp.tile([P, QC], F32, tag="pp")
                    for et in range(ET):
                        nc.tensor.matmul(psum[:], w_sb[:, et, ft * P:(ft + 1) * P],
                                         xc[:, et, :],
                                         start=(et == 0), stop=(et == ET - 1))
                    if qk_copy == "scalar":
                        nc.scalar.copy(dst[:, ft, sc * QC:(sc + 1) * QC], psum[:])
                    else:
                        nc.vector.tensor_copy(dst[:, ft, sc * QC:(sc + 1) * QC], psum[:])

                def v_unit(wave, st):
                    si = st % KT_PER_QC
                    psum = ps_pp.tile([P, QC], F32, tag="pp")
                    for et in range(ET):
                        nc.tensor.matmul(psum[:], wave["xv"][:, et, si * P:(si + 1) * P],
                                         wv_sb[:, et, :],
                                         start=(et == 0), stop=(et == ET - 1))
                    nc.vector.tensor_copy(
                        vh_sb[:, st, :, 0:D],
                        psum[:].rearrange("p (h d) -> p h d", h=H))

                def p3_unit(jt, qc):
                    psum = ps_pp.tile([P, QC], F32, tag="pp")
                    for ft in range(FT):
                        nc.tensor.matmul(psum[:], wo_sb[:, ft, jt * P:(jt + 1) * P],
                                         xts_sb[:, ft, qc * QC:(qc + 1) * QC],
                                         start=(ft == 0), stop=(ft == FT - 1))
                    ot = otp.tile([P, QC], F32, tag="ot")
                    nc.vector.tensor_copy(ot[:], psum[:])
                    nc.sync.dma_start(
                        outT[jt * P:(jt + 1) * P, qc * QC:(qc + 1) * QC], ot[:])

                def head_block(h, qc):
                    ft, fo = h // 2, (h % 2) * D
                    xt_t = ps_xt.tile([D + 1, QC], F32, tag="xt")
                    pend = []

                    def flush_one():
                        k0, n, at_t = pend.pop(0)
                        for j in range(n):
                            kt = k0 + j
                            nc.tensor.matmul(xt_t[:], vh_sb[:, kt, h, :],
                                             at_t[:, j * QC:(j + 1) * QC],
                                             start=(kt == 0), stop=(kt == 4 * qc + 3))

                    for (k0, n, mcol) in _plan_groups(qc):
                        sc_t = ps_sc.tile([P, 3 * QC], F32, tag="sc")
                        for j in range(n):
                            kt = k0 + j
                            nc.tensor.matmul(
                                sc_t[:, j * QC:(j + 1) * QC],
                                kh_sb[fo:fo + D, ft, kt * P:(kt + 1) * P],
                                qh_sb[fo:fo + D, ft, qc * QC:(qc + 1) * QC],
                                start=True, stop=True)
                        w = n * QC
                        at_t = attnp.tile([P, 3 * QC] if n > 1 else [P, QC], BF16,
                                          tag=("at3" if n > 1 else "at1"))
                        nc.scalar.activation(at_t[:, 0:w], sc_t[:, 0:w],
                                             mybir.ActivationFunctionType.Exp,
                                             scale=0.125)
                        if mcol is not None:
                            nc.vector.tensor_mul(at_t[:, 0:w], at_t[:, 0:w],
                                                 dmask_sb[:, mcol:mcol + w])
                        pend.append((k0, n, at_t))
                        if len(pend) > lag:
                            flush_one()
                    while pend:
                        flush_one()
                    # normalize
                    recip = smalls.tile([1, QC], F32, tag="recip")
                    nc.vector.reciprocal(recip[:], xt_t[D:D + 1, :])
                    rb = smalls.tile([D, QC], F32, tag="rb")
                    nc.gpsimd.partition_broadcast(rb[:], recip[0:1, :])
                    nc.vector.tensor_mul(
                        xts_sb[fo:fo + D, ft, qc * QC:(qc + 1) * QC],
                        xt_t[0:D, :], rb[:])

                # ---- lead-in ----
                cur = dma_wave(0)
                qk_unit(cur, "q", 0, 0)
                qk_unit(cur, "k", 0, 0)
                for st in range(KT_PER_QC):
                    v_unit(cur, st)

                # ---- waves ----
                for qc in range(NQC):
                    nxt = dma_wave(qc + 1) if qc + 1 < NQC else None
                    proj_fill = ([(s, f) for f in range(FT) for s in ("q", "k")]
                                 if nxt else [])
                    v_fill = ([4 * (qc + 1) + i for i in range(KT_PER_QC)]
                              if nxt else [])
                    p3_fill = [(jt, qc - 1) for jt in range(ET)] if qc >= 1 else []
                    for h in range(H):
                        if qc == 0 and h >= 2 and h % 2 == 0:
                            qk_unit(cur, "q", h // 2, 0)
                            qk_unit(cur, "k", h // 2, 0)
                        head_block(h, qc)
                        if nxt:
                            s, f = proj_fill[h]
                            qk_unit(nxt, s, f, qc + 1)
                            if h % 2 == 1:
                                v_unit(nxt, v_fill[h // 2])
                        if p3_fill:
                            jt, qcp = p3_fill[h]
                            p3_unit(jt, qcp)
                    cur = nxt

                # ---- tail ----
                for jt in range(ET):
                    p3_unit(jt, NQC - 1)

            if niter is None:
                body()
            else:
                assert niter % unroll == 0
                with tc.For_i(0, niter // unroll, 1):
                    for _ in range(unroll):
                        body()

    nc.compile()
    return nc


def build_nc3(niter=None, lag=2, at_bufs3=4, pp_bufs=2, exact_recip=True,
              split_exp=False, serial_scores=False):
    """Causal-only v3.  Per (qc, head-pair) block: at each kt step, two
    row-tiled concurrent K=64 scores matmuls (rows 0-63 / 64-127) write one
    2-bank PSUM pair-tile, a single FD=1024 exp converts both, a duplicated
    mask handles the diagonal, and two attn@V matmuls accumulate per-head
    xt.  Projection + output-projection units are interleaved as PE filler."""
    nc = bacc.Bacc("TRN2", target_bir_lowering=False, debug=False,
                   enable_asserts=True, num_devices=8)

    qT = nc.dram_tensor("qT", [E, S], BF16, kind="ExternalInput").ap()
    kT = nc.dram_tensor("kT", [E, S], BF16, kind="ExternalInput").ap()
    vT = nc.dram_tensor("vT", [E, S], BF16, kind="ExternalInput").ap()
    wqT = nc.dram_tensor("wqT", [E, F], BF16, kind="ExternalInput").ap()
    wkT = nc.dram_tensor("wkT", [E, F], BF16, kind="ExternalInput").ap()
    wvT = nc.dram_tensor("wvT", [E, F], BF16, kind="ExternalInput").ap()
    woT = nc.dram_tensor("woT", [F, E], BF16, kind="ExternalInput").ap()
    dmask2 = nc.dram_tensor("dmask2", [P, 8 * QC], BF16, kind="ExternalInput").ap()
    outT = nc.dram_tensor("outT", [E, S], F32, kind="ExternalOutput").ap()

    qT3 = qT.rearrange("(o p) s -> p o s", p=P)
    kT3 = kT.rearrange("(o p) s -> p o s", p=P)
    vT3 = vT.rearrange("(o p) s -> p o s", p=P)

    with tile.TileContext(nc) as tc:
        import contextlib
        with contextlib.ExitStack() as ctx:
            persist = ctx.enter_context(tc.tile_pool(name="persist", bufs=1))
            streams = ctx.enter_context(tc.tile_pool(name="streams", bufs=2))
            otp = ctx.enter_context(tc.tile_pool(name="otp", bufs=3))
            attnp = ctx.enter_context(tc.tile_pool(name="attnp", bufs=at_bufs3))
            smalls = ctx.enter_context(tc.tile_pool(name="smalls", bufs=3))
            ps_sc = ctx.enter_context(tc.tile_pool(name="ps_sc", bufs=2, space="PSUM"))
            ps_xt = ctx.enter_context(tc.tile_pool(name="ps_xt", bufs=2, space="PSUM"))
            ps_pp = ctx.enter_context(tc.tile_pool(name="ps_pp", bufs=pp_bufs, space="PSUM"))

            wq_sb = persist.tile([P, ET, F], BF16, tag="wq")
            wk_sb = persist.tile([P, ET, F], BF16, tag="wk")
            wv_sb = persist.tile([P, ET, F], BF16, tag="wv")
            wo_sb = persist.tile([P, FT, E], BF16, tag="wo")
            dm_sb = persist.tile([P, 8 * QC], BF16, tag="dmask2")
            nc.sync.dma_start(wq_sb[:], wqT.rearrange("(o p) f -> p o f", p=P))
            nc.sync.dma_start(wk_sb[:], wkT.rearrange("(o p) f -> p o f", p=P))
            nc.sync.dma_start(wv_sb[:], wvT.rearrange("(o p) f -> p o f", p=P))
            nc.sync.dma_start(wo_sb[:], woT.rearrange("(o p) e -> p o e", p=P))
            nc.sync.dma_start(dm_sb[:], dmask2[:])

            qh_sb = persist.tile([P, FT, S], BF16, tag="qh")
            kh_sb = persist.tile([P, FT, S], BF16, tag="kh")
            vh_sb = persist.tile([P, ST, H, D + 1], BF16, tag="vh")
            xts_sb = persist.tile([P, FT, S], BF16, tag="xts")

            def body():
                nc.vector.memset(vh_sb[:, :, :, D:D + 1], 1.0)

                def dma_wave(sc):
                    w = {}
                    for tag, src3 in (("xq", qT3), ("xk", kT3), ("xv", vT3)):
                        t = streams.tile([P, ET, QC], BF16, tag=tag)
                        nc.sync.dma_start(t[:], src3[:, :, sc * QC:(sc + 1) * QC])
                        w[tag] = t
                    return w

                def qk_unit(wave, which, ft, sc):
                    w_sb, dst, xc = ((wq_sb, qh_sb, wave["xq"]) if which == "q"
                                     else (wk_sb, kh_sb, wave["xk"]))
                    psum = ps_pp.tile([P, QC], F32, tag="pp")
                    for et in range(ET):
                        nc.tensor.matmul(psum[:], w_sb[:, et, ft * P:(ft + 1) * P],
                                         xc[:, et, :],
                                         start=(et == 0), stop=(et == ET - 1))
                    nc.vector.tensor_copy(dst[:, ft, sc * QC:(sc + 1) * QC], psum[:])

                def v_unit(wave, st):
                    si = st % KT_PER_QC
                    psum = ps_pp.tile([P, QC], F32, tag="pp")
                    for et in range(ET):
                        nc.tensor.matmul(psum[:], wave["xv"][:, et, si * P:(si + 1) * P],
                                         wv_sb[:, et, :],
                                         start=(et == 0), stop=(et == ET - 1))
                    nc.vector.tensor_copy(
                        vh_sb[:, st, :, 0:D],
                        psum[:].rearrange("p (h d) -> p h d", h=H))

                def p3_unit(jt, qc):
                    psum = ps_pp.tile([P, QC], F32, tag="pp")
                    for ft in range(FT):
                        nc.tensor.matmul(psum[:], wo_sb[:, ft, jt * P:(jt + 1) * P],
                                         xts_sb[:, ft, qc * QC:(qc + 1) * QC],
                                         start=(ft == 0), stop=(ft == FT - 1))
                    ot = otp.tile([P, QC], F32, tag="ot")
                    nc.vector.tensor_copy(ot[:], psum[:])
                    nc.sync.dma_start(
                        outT[jt * P:(jt + 1) * P, qc * QC:(qc + 1) * QC], ot[:])

                def normalize(xt_t, h, qc):
                    ft, fo = h // 2, (h % 2) * D
                    recip = smalls.tile([1, QC], F32, tag="recip")
                    if exact_recip:
                        nc.vector.reciprocal(recip[:], xt_t[D:D + 1, :])
                    else:
                        nc.vector.reciprocal_approx_fast(recip[:], xt_t[D:D + 1, :])
                    rb = smalls.tile([D, QC], F32, tag="rb")
                    nc.gpsimd.partition_broadcast(rb[:], recip[0:1, :])
                    nc.vector.tensor_mul(
                        xts_sb[fo:fo + D, ft, qc * QC:(qc + 1) * QC],
                        xt_t[0:D, :], rb[:])

                def pair_block(p, qc):
                    h0, h1 = 2 * p, 2 * p + 1
                    ft = p
                    nkt = 4 * qc + 4
                    xt0 = ps_xt.tile([D + 1, QC], F32, tag="xt")
                    xt1 = ps_xt.tile([D + 1, QC], F32, tag="xt")
                    pend = []

                    def flush_one():
                        kt, at_t = pend.pop(0)
                        st, sp = (kt == 0), (kt == nkt - 1)
                        nc.tensor.matmul(xt0[:], vh_sb[:, kt, h0, :],
                                         at_t[:, 0:QC], start=st, stop=sp)
                        nc.tensor.matmul(xt1[:], vh_sb[:, kt, h1, :],
                                         at_t[:, QC:2 * QC], start=st, stop=sp)

                    for kt in range(nkt):
                        sc_t = ps_sc.tile([P, 2 * QC], F32, tag="sc")
                        for fo, cs in ((0, slice(0, QC)), (D, slice(QC, 2 * QC))):
                            nc.tensor.matmul(
                                sc_t[:, cs],
                                kh_sb[fo:fo + D, ft, kt * P:(kt + 1) * P],
                                qh_sb[fo:fo + D, ft, qc * QC:(qc + 1) * QC],
                                start=True, stop=True)
                        at_t = attnp.tile([P, 2 * QC], BF16, tag="at")
                        if split_exp:
                            for cs in (slice(0, QC), slice(QC, 2 * QC)):
                                nc.scalar.activation(at_t[:, cs], sc_t[:, cs],
                                                     mybir.ActivationFunctionType.Exp,
                                                     scale=0.125)
                        else:
                            nc.scalar.activation(at_t[:], sc_t[:],
                                                 mybir.ActivationFunctionType.Exp,
                                                 scale=0.125)
                        jd = kt - 4 * qc
                        if jd >= 0:
                            nc.vector.tensor_mul(
                                at_t[:], at_t[:],
                                dm_sb[:, jd * 2 * QC:(jd + 1) * 2 * QC])
                        pend.append((kt, at_t))
                        if len(pend) > lag:
                            flush_one()
                    while pend:
                        flush_one()
                    normalize(xt0, h0, qc)
                    normalize(xt1, h1, qc)

                # ---- lead-in ----
                cur = dma_wave(0)
                qk_unit(cur, "q", 0, 0)
                qk_unit(cur, "k", 0, 0)
                for st in range(KT_PER_QC):
                    v_unit(cur, st)

                # ---- waves ----
                for qc in range(NQC):
                    nxt = dma_wave(qc + 1) if qc + 1 < NQC else None
                    for p in range(FT):
                        if qc == 0 and p >= 1:
                            qk_unit(cur, "q", p, 0)
                            qk_unit(cur, "k", p, 0)
                        pair_block(p, qc)
                        if nxt:
                            qk_unit(nxt, "q", p, qc + 1)
                            qk_unit(nxt, "k", p, qc + 1)
                            v_unit(nxt, 4 * (qc + 1) + p)
                        if qc >= 1:
                            p3_unit(2 * p, qc - 1)
                            p3_unit(2 * p + 1, qc - 1)
                    cur = nxt

                # ---- tail ----
                for jt in range(ET):
                    p3_unit(jt, NQC - 1)

            if niter is None:
                body()
            else:
                with tc.For_i(0, niter, 1):
                    body()

    nc.compile()
    return nc


def _host_prep(q, k, v, mask, w_q, w_k, w_v, w_o):
    """Shard + transpose inputs on the host.  Returns (in_maps, causal)."""
    tril = np.tril(np.ones((S, S), dtype=mask.dtype))
    causal = all(np.array_equal(np.asarray(mask[b, 0]), tril) for b in range(B))

    stair = (np.arange(2 * QC)[None, :] >= (np.arange(P)[:, None] + QC))
    stair = stair.astype(NPBF16)

    # v2 diag mask: 4 concatenated [P, QC] tiles, tile jd valid iff q >= p + 128*jd
    dmask = np.concatenate(
        [(np.arange(QC)[None, :] >= (np.arange(P)[:, None] + P * jd))
         for jd in range(4)], axis=1).astype(NPBF16)
    # v3 pair mask: same, but each jd tile duplicated side-by-side for the
    # two heads of a pair sharing one [P, 2*QC] at-tile
    dmask2 = np.concatenate(
        [np.tile((np.arange(QC)[None, :] >= (np.arange(P)[:, None] + P * jd)), (1, 2))
         for jd in range(4)], axis=1).astype(NPBF16)

    w_q = np.asarray(w_q, dtype=np.float32)
    w_k = np.asarray(w_k, dtype=np.float32)
    w_v = np.asarray(w_v, dtype=np.float32)
    w_o = np.asarray(w_o, dtype=np.float32)

    in_maps = []
    for core in range(8):
        b, g = divmod(core, 2)
        rows = slice(g * F, (g + 1) * F)
        m = {
            "qT": np.ascontiguousarray(np.asarray(q[b], np.float32).T).astype(NPBF16),
            "kT": np.ascontiguousarray(np.asarray(k[b], np.float32).T).astype(NPBF16),
            "vT": np.ascontiguousarray(np.asarray(v[b], np.float32).T).astype(NPBF16),
            "wqT": np.ascontiguousarray(w_q[rows, :].T).astype(NPBF16),
            "wkT": np.ascontiguousarray(w_k[rows, :].T).astype(NPBF16),
            "wvT": np.ascontiguousarray(w_v[rows, :].T).astype(NPBF16),
            "woT": np.ascontiguousarray(w_o[:, rows].T).astype(NPBF16),
            "stair": stair,
            "dmask": dmask,
            "dmask2": dmask2,
        }
        if not causal:
            m["maskT"] = np.ascontiguousarray(
                np.asarray(mask[b, 0], np.float32).T).astype(NPBF16)
        in_maps.append(m)
    return in_maps, causal


_NC_CACHE: dict = {}


def kernel(q, k, v, mask, w_q, w_k, w_v, w_o):
    in_maps, causal = _host_prep(q, k, v, mask, w_q, w_k, w_v, w_o)
    nc = _NC_CACHE.get(causal)
    if nc is None:
        nc = build_nc3() if causal else build_nc(causal)
        _NC_CACHE[causal] = nc
    res = bass_utils.run_bass_kernel_spmd(nc, in_maps, core_ids=list(range(8)))
    out = np.empty((B, S, E), dtype=np.float32)
    for b in range(B):
        out[b] = (res.results[2 * b]["outT"] + res.results[2 * b + 1]["outT"]).T
    return out



# revision 25
# speedup vs baseline: 1.3962x; 1.2449x over previous
"""Multi-head attention block kernel for Trainium2, sharded over 8 NeuronCores.

Sharding: batch (4) x head-group (2 groups of 8 heads) -> 8 cores.
Each core computes, for one batch b and one half of the heads:
  qh/kh/vh projections (columns of w_q/w_k/w_v for its heads),
  causal attention for its 8 heads, and a partial output projection
  (rows of w_o^T for its heads).  Host sums the two partial outputs per
  batch and transposes back.

On-chip layout is feature-major ("transposed"): activations live as
[feature, seq] so every matmul contraction dim is on partitions and no
on-chip transposes are needed.  Host pre-transposes q/k/v and the
weight slices, and post-transposes the output.

Matmuls run in bf16 (fp32 matmul is 4x slower on TRN2); accumulation is
fp32 in PSUM.  Softmax denominators come for free from an extra ones
column appended to each V tile (row 64 of the attn@V accumulator is the
sum of exp scores).
"""

import sys

sys.path.insert(0, "/opt/trn_rl_repo")

import numpy as np
import ml_dtypes

import concourse.bacc as bacc
import concourse.mybir as mybir
import concourse.tile as tile
from concourse import bass_utils

B = 4
S = 2048
E = 1024
HEADS = 16
D = 64
H = 8            # heads per core
F = H * D        # 512 local head features
P = 128
ET = E // P      # 8 e-tiles
FT = F // P      # 4 f-tiles
ST = S // P      # 16 s-tiles
QC = 512         # q-chunk width
NQC = S // QC    # 4 q-chunks
KT_PER_QC = QC // P  # 4 k-tiles per q-chunk

BF16 = mybir.dt.bfloat16
F32 = mybir.dt.float32
NPBF16 = ml_dtypes.bfloat16


def build_nc(causal: bool, niter: int | None = None, phases=(1, 2, 3), no_norm=False, no_exp=False,
             p1_wide=4, p3_wide=4, xtlag=2, sc_bufs=0, ps_bufs=4, at_bufs=12, old_p2=False):
    """Build the per-core Bass program.  If niter is given, wrap the body in a
    For_i timing loop (used by test.py to measure HW time)."""
    nc = bacc.Bacc("TRN2", target_bir_lowering=False, debug=False,
                   enable_asserts=True, num_devices=8)

    qT = nc.dram_tensor("qT", [E, S], BF16, kind="ExternalInput").ap()
    kT = nc.dram_tensor("kT", [E, S], BF16, kind="ExternalInput").ap()
    vT = nc.dram_tensor("vT", [E, S], BF16, kind="ExternalInput").ap()
    wqT = nc.dram_tensor("wqT", [E, F], BF16, kind="ExternalInput").ap()
    wkT = nc.dram_tensor("wkT", [E, F], BF16, kind="ExternalInput").ap()
    wvT = nc.dram_tensor("wvT", [E, F], BF16, kind="ExternalInput").ap()
    woT = nc.dram_tensor("woT", [F, E], BF16, kind="ExternalInput").ap()
    stair = nc.dram_tensor("stair", [P, 2 * QC], BF16, kind="ExternalInput").ap()
    if not causal:
        maskT = nc.dram_tensor("maskT", [S, S], BF16, kind="ExternalInput").ap()
    outT = nc.dram_tensor("outT", [E, S], F32, kind="ExternalOutput").ap()

    qT3 = qT.rearrange("(o p) s -> p o s", p=P)
    kT3 = kT.rearrange("(o p) s -> p o s", p=P)
    vT3 = vT.rearrange("(o p) s -> p o s", p=P)
    if not causal:
        maskT3 = maskT.rearrange("(o p) s -> p o s", p=P)

    with tile.TileContext(nc) as tc:
        import contextlib
        with contextlib.ExitStack() as ctx:
            persist = ctx.enter_context(tc.tile_pool(name="persist", bufs=1))
            streams = ctx.enter_context(tc.tile_pool(name="streams", bufs=6))
            attnp = ctx.enter_context(tc.tile_pool(name="attnp", bufs=at_bufs))
            smalls = ctx.enter_context(tc.tile_pool(name="smalls", bufs=3))
            ps1 = ctx.enter_context(tc.tile_pool(name="ps1", bufs=ps_bufs, space="PSUM"))
            if sc_bufs:
                ps_sc = ctx.enter_context(tc.tile_pool(name="ps_sc", bufs=sc_bufs, space="PSUM"))
            ps_xt = ctx.enter_context(tc.tile_pool(name="ps_xt", bufs=1, space="PSUM"))
            def sc_tile():
                if sc_bufs:
                    return ps_sc.tile([P, QC], F32, tag="sc", name="scp")
                return ps1.tile([P, QC], F32, tag="ps", name="scp")

            # Weights + constants: loaded once, outside the timing loop.
            wq_sb = persist.tile([P, ET, F], BF16, tag="wq")
            wk_sb = persist.tile([P, ET, F], BF16, tag="wk")
            wv_sb = persist.tile([P, ET, F], BF16, tag="wv")
            wo_sb = persist.tile([P, FT, E], BF16, tag="wo")
            stair_sb = persist.tile([P, 2 * QC], BF16, tag="stair")
            nc.sync.dma_start(wq_sb[:], wqT.rearrange("(o p) f -> p o f", p=P))
            nc.sync.dma_start(wk_sb[:], wkT.rearrange("(o p) f -> p o f", p=P))
            nc.sync.dma_start(wv_sb[:], wvT.rearrange("(o p) f -> p o f", p=P))
            nc.sync.dma_start(wo_sb[:], woT.rearrange("(o p) e -> p o e", p=P))
            nc.sync.dma_start(stair_sb[:], stair[:])

            # Persistent activations (bf16): projections and attention outputs.
            qh_sb = persist.tile([P, FT, S], BF16, tag="qh")    # [f, ft, s]
            kh_sb = persist.tile([P, FT, S], BF16, tag="kh")
            vh_sb = persist.tile([P, ST, H, D + 1], BF16, tag="vh")  # ones col at d=64
            xts_sb = persist.tile([P, FT, S], BF16, tag="xts")

            def body():
                run1 = 1 in phases
                run2 = 2 in phases
                run3 = 3 in phases
                if not run1:
                    nc.vector.memset(qh_sb[:, :, 0:1], 0.5)
                    nc.vector.memset(kh_sb[:, :, 0:1], 0.5)
                    nc.vector.memset(vh_sb[:, :, :, 0:1], 0.5)
                if not run2 and run3:
                    nc.vector.memset(xts_sb[:, :, 0:1], 0.5)
                # ---- Phase 1a: q/k projections -> qh/kh (feature-major) ----
                # Weight-stationary: for each (ft, et) weight tile, stream all
                # 4 s-chunks into 4 accumulating PSUMs so LDWEIGHTS happens
                # once per 4 matmuls.
                for src3, w_sb, dst in ((qT3, wq_sb, qh_sb), (kT3, wk_sb, kh_sb)) if run1 else ():
                    xcs = []
                    for sc in range(NQC):
                        xc = streams.tile([P, ET, QC], BF16, tag="xc")
                        nc.sync.dma_start(xc[:], src3[:, :, sc * QC:(sc + 1) * QC])
                        xcs.append(xc)
                    for ft in range(FT):
                        if p1_wide > 1:
                            for g0 in range(0, NQC, p1_wide):
                                gs = list(range(g0, min(NQC, g0 + p1_wide)))
                                psums = [ps1.tile([P, QC], F32, tag="ps", name=f"pp{sc}")
                                         for sc in gs]
                                for et in range(ET):
                                    for i, sc in enumerate(gs):
                                        nc.tensor.matmul(
                                            psums[i][:],
                                            w_sb[:, et, ft * P:(ft + 1) * P],
                                            xcs[sc][:, et, :],
                                            start=(et == 0), stop=(et == ET - 1))
                                for i, sc in enumerate(gs):
                                    nc.vector.tensor_copy(
                                        dst[:, ft, sc * QC:(sc + 1) * QC], psums[i][:])
                        else:
                            for sc in range(NQC):
                                psum = ps1.tile([P, QC], F32, tag="ps", name="pp")
                                for et in range(ET):
                                    nc.tensor.matmul(
                                        psum[:],
                                        w_sb[:, et, ft * P:(ft + 1) * P],
                                        xcs[sc][:, et, :],
                                        start=(et == 0), stop=(et == ET - 1))
                                nc.vector.tensor_copy(
                                    dst[:, ft, sc * QC:(sc + 1) * QC], psum[:])

                # ---- Phase 1b: v projection -> vh (seq-major) + ones column ----
                nc.vector.memset(vh_sb[:, :, :, D:D + 1], 1.0)
                for sc in range(NQC) if run1 else ():
                    xc = streams.tile([P, ET, QC], BF16, tag="xc")
                    nc.sync.dma_start(xc[:], vT3[:, :, sc * QC:(sc + 1) * QC])
                    for si in range(KT_PER_QC):
                        st = sc * KT_PER_QC + si
                        psum = ps1.tile([P, QC], F32, tag="ps")
                        for et in range(ET):
                            nc.tensor.matmul(
                                psum[:],
                                xc[:, et, si * P:(si + 1) * P],
                                wv_sb[:, et, :],
                                start=(et == 0), stop=(et == ET - 1))
                        nc.vector.tensor_copy(
                            vh_sb[:, st, :, 0:D],
                            psum[:].rearrange("p (h d) -> p h d", h=H))

                # ---- Phase 2: attention ----
                # Causal path: kt-outer so the stationary operands (K tile for
                # scores, V tile for attn@V) are each loaded once per (h, kt)
                # and streamed over all valid q-chunks (LDWEIGHTS amortization;
                # weight switches cost ~250ns on PE).  Needs one xt accumulator
                # per q-chunk (4 PSUM banks).
                def normalize(xt_psum, h, qc):
                    ft, fo = h // 2, (h % 2) * D
                    if no_norm:
                        nc.vector.tensor_copy(
                            xts_sb[fo:fo + D, ft, qc * QC:(qc + 1) * QC],
                            xt_psum[0:D, :])
                    else:
                        recip = smalls.tile([1, QC], F32, tag="recip")
                        nc.vector.reciprocal(recip[:], xt_psum[D:D + 1, :])
                        rb = smalls.tile([D, QC], F32, tag="rb")
                        nc.gpsimd.partition_broadcast(rb[:], recip[0:1, :])
                        nc.vector.tensor_mul(
                            xts_sb[fo:fo + D, ft, qc * QC:(qc + 1) * QC],
                            xt_psum[0:D, :], rb[:])

                def emit_exp(at, sc_psum, kt, qc, mc):
                    if no_exp:
                        nc.vector.tensor_copy(at[:], sc_psum[:])
                    else:
                        nc.scalar.activation(at[:], sc_psum[:],
                                             mybir.ActivationFunctionType.Exp,
                                             scale=0.125)
                    if causal:
                        if kt // KT_PER_QC == qc:
                            off = kt * P - qc * QC
                            nc.vector.tensor_mul(
                                at[:], at[:], stair_sb[:, QC - off:2 * QC - off])
                    else:
                        nc.vector.tensor_mul(at[:], at[:], mc[:, kt, :])

                if run2 and causal and not old_p2:
                    for h in range(H):
                        ft, fo = h // 2, (h % 2) * D
                        xt_psums = [ps_xt.tile([D + 1, QC], F32, tag=f"xt{qc}", name=f"xt{qc}")
                                    for qc in range(NQC)]
                        pend = []   # [(kt, qc, at)] generations awaiting attn@V

                        def flush_xt(gen):
                            for kt, qc, at in gen:
                                nc.tensor.matmul(
                                    xt_psums[qc][:],
                                    vh_sb[:, kt, h, :],
                                    at[:],
                                    start=(kt == 0),
                                    stop=(kt == (qc + 1) * KT_PER_QC - 1))

                        XTLAG = xtlag
                        for kt in range(ST):
                            qcs = [qc for qc in range(NQC)
                                   if kt < (qc + 1) * KT_PER_QC]
                            nxt = []
                            for qc in qcs:
                                sc_psum = sc_tile()
                                nc.tensor.matmul(
                                    sc_psum[:],
                                    kh_sb[fo:fo + D, ft, kt * P:(kt + 1) * P],
                                    qh_sb[fo:fo + D, ft, qc * QC:(qc + 1) * QC],
                                    start=True, stop=True)
                                at = attnp.tile([P, QC], BF16, tag="at")
                                emit_exp(at, sc_psum, kt, qc, None)
                                nxt.append((kt, qc, at))
                            pend.append(nxt)
                            if len(pend) > XTLAG:
                                flush_xt(pend.pop(0))
                        for gen in pend:
                            flush_xt(gen)
                        for qc in range(NQC):
                            normalize(xt_psums[qc], h, qc)

                elif run2:
                    # general-mask path: qc-outer, mask tiles streamed per qc.
                    # (also used as the old_p2 comparison structure for causal)
                    for qc in range(NQC):
                        if causal:
                            mc = None
                            ktm = (qc + 1) * KT_PER_QC
                        else:
                            mc = streams.tile([P, ST, QC], BF16, tag="mc")
                            nc.sync.dma_start(mc[:], maskT3[:, :, qc * QC:(qc + 1) * QC])
                            ktm = ST
                        for h in range(H):
                            ft, fo = h // 2, (h % 2) * D
                            xt_psum = ps_xt.tile([D + 1, QC], F32, tag="xt0")
                            at_tiles = [None] * ktm

                            def emit_sc(kt):
                                sc_psum = sc_tile()
                                nc.tensor.matmul(
                                    sc_psum[:],
                                    kh_sb[fo:fo + D, ft, kt * P:(kt + 1) * P],
                                    qh_sb[fo:fo + D, ft, qc * QC:(qc + 1) * QC],
                                    start=True, stop=True)
                                at = attnp.tile([P, QC], BF16, tag="at")
                                emit_exp(at, sc_psum, kt, qc, mc)
                                at_tiles[kt] = at

                            def emit_xt(kt):
                                nc.tensor.matmul(
                                    xt_psum[:],
                                    vh_sb[:, kt, h, :],
                                    at_tiles[kt][:],
                                    start=(kt == 0), stop=(kt == ktm - 1))

                            PIPE = 2
                            for kt in range(ktm):
                                emit_sc(kt)
                                if kt >= PIPE:
                                    emit_xt(kt - PIPE)
                            for kt in range(max(0, ktm - PIPE), ktm):
                                emit_xt(kt)
                            normalize(xt_psum, h, qc)

                # ---- Phase 3: output projection (partial over local heads) ----
                for jt in range(ET) if run3 else ():
                    if p3_wide > 1:
                        psums = [ps1.tile([P, QC], F32, tag="ps", name=f"po{qc}")
                                 for qc in range(NQC)]
                        for ft in range(FT):
                            for qc in range(NQC):
                                nc.tensor.matmul(
                                    psums[qc][:],
                                    wo_sb[:, ft, jt * P:(jt + 1) * P],
                                    xts_sb[:, ft, qc * QC:(qc + 1) * QC],
                                    start=(ft == 0), stop=(ft == FT - 1))
                        for qc in range(NQC):
                            ot = streams.tile([P, QC], F32, tag="ot")
                            nc.vector.tensor_copy(ot[:], psums[qc][:])
                            nc.sync.dma_start(
                                outT[jt * P:(jt + 1) * P, qc * QC:(qc + 1) * QC],
                                ot[:])
                    else:
                        for qc in range(NQC):
                            psum = ps1.tile([P, QC], F32, tag="ps", name="po")
                            for ft in range(FT):
                                nc.tensor.matmul(
                                    psum[:],
                                    wo_sb[:, ft, jt * P:(jt + 1) * P],
                                    xts_sb[:, ft, qc * QC:(qc + 1) * QC],
                                    start=(ft == 0), stop=(ft == FT - 1))
                            ot = streams.tile([P, QC], F32, tag="ot")
                            nc.vector.tensor_copy(ot[:], psum[:])
                            nc.sync.dma_start(
                                outT[jt * P:(jt + 1) * P, qc * QC:(qc + 1) * QC],
                                ot[:])

            if niter is None:
                body()
            else:
                with tc.For_i(0, niter, 1):
                    body()

    nc.compile()
    return nc


def _plan_groups(qc):
    """kt-tile groups for one (h, qc) block: non-diag groups of <=3 (no mask),
    then the 4 diagonal tiles as [3, 1] with fixed mask-table slices."""
    nd = 4 * qc
    groups = []
    k0 = 0
    while k0 < nd:
        n = min(3, nd - k0)
        groups.append((k0, n, None))
        k0 += n
    groups.append((nd, 3, 0))            # diag tiles jd=0..2 -> dmask cols [0, 1536)
    groups.append((nd + 3, 1, 3 * QC))   # diag tile jd=3 -> dmask cols [1536, 2048)
    return groups


def build_nc2(niter=None, lag=1, at3_bufs=3, qk_copy="scalar", unroll=1):
    """Causal-only v2: qc-major waves, batched exp over 3-bank PSUM groups,
    proj/out-proj units interleaved into the attention stream as PE filler."""
    nc = bacc.Bacc("TRN2", target_bir_lowering=False, debug=False,
                   enable_asserts=True, num_devices=8)

    qT = nc.dram_tensor("qT", [E, S], BF16, kind="ExternalInput").ap()
    kT = nc.dram_tensor("kT", [E, S], BF16, kind="ExternalInput").ap()
    vT = nc.dram_tensor("vT", [E, S], BF16, kind="ExternalInput").ap()
    wqT = nc.dram_tensor("wqT", [E, F], BF16, kind="ExternalInput").ap()
    wkT = nc.dram_tensor("wkT", [E, F], BF16, kind="ExternalInput").ap()
    wvT = nc.dram_tensor("wvT", [E, F], BF16, kind="ExternalInput").ap()
    woT = nc.dram_tensor("woT", [F, E], BF16, kind="ExternalInput").ap()
    dmask = nc.dram_tensor("dmask", [P, 4 * QC], BF16, kind="ExternalInput").ap()
    outT = nc.dram_tensor("outT", [E, S], F32, kind="ExternalOutput").ap()

    qT3 = qT.rearrange("(o p) s -> p o s", p=P)
    kT3 = kT.rearrange("(o p) s -> p o s", p=P)
    vT3 = vT.rearrange("(o p) s -> p o s", p=P)

    with tile.TileContext(nc) as tc:
        import contextlib
        with contextlib.ExitStack() as ctx:
            persist = ctx.enter_context(tc.tile_pool(name="persist", bufs=1))
            streams = ctx.enter_context(tc.tile_pool(name="streams", bufs=2))
            otp = ctx.enter_context(tc.tile_pool(name="otp", bufs=3))
            attnp = ctx.enter_context(tc.tile_pool(name="attnp", bufs=at3_bufs))
            smalls = ctx.enter_context(tc.tile_pool(name="smalls", bufs=3))
            ps_sc = ctx.enter_context(tc.tile_pool(name="ps_sc", bufs=2, space="PSUM"))
            ps_xt = ctx.enter_context(tc.tile_pool(name="ps_xt", bufs=1, space="PSUM"))
            ps_pp = ctx.enter_context(tc.tile_pool(name="ps_pp", bufs=1, space="PSUM"))

            wq_sb = persist.tile([P, ET, F], BF16, tag="wq")
            wk_sb = persist.tile([P, ET, F], BF16, tag="wk")
            wv_sb = persist.tile([P, ET, F], BF16, tag="wv")
            wo_sb = persist.tile([P, FT, E], BF16, tag="wo")
            dmask_sb = persist.tile([P, 4 * QC], BF16, tag="dmask")
            nc.sync.dma_start(wq_sb[:], wqT.rearrange("(o p) f -> p o f", p=P))
            nc.sync.dma_start(wk_sb[:], wkT.rearrange("(o p) f -> p o f", p=P))
            nc.sync.dma_start(wv_sb[:], wvT.rearrange("(o p) f -> p o f", p=P))
            nc.sync.dma_start(wo_sb[:], woT.rearrange("(o p) e -> p o e", p=P))
            nc.sync.dma_start(dmask_sb[:], dmask[:])

            qh_sb = persist.tile([P, FT, S], BF16, tag="qh")
            kh_sb = persist.tile([P, FT, S], BF16, tag="kh")
            vh_sb = persist.tile([P, ST, H, D + 1], BF16, tag="vh")
            xts_sb = persist.tile([P, FT, S], BF16, tag="xts")

            def body():
                nc.vector.memset(vh_sb[:, :, :, D:D + 1], 1.0)

                def dma_wave(sc):
                    w = {}
                    for tag, src3 in (("xq", qT3), ("xk", kT3), ("xv", vT3)):
                        t = streams.tile([P, ET, QC], BF16, tag=tag)
                        nc.sync.dma_start(t[:], src3[:, :, sc * QC:(sc + 1) * QC])
                        w[tag] = t
                    return w

                def qk_unit(wave, which, ft, sc):
                    w_sb, dst, xc = ((wq_sb, qh_sb, wave["xq"]) if which == "q"
                                     else (wk_sb, kh_sb, wave["xk"]))
                    psum = ps_pp.tile([P, QC], F32, tag="pp")
                    for et in range(ET):
                        nc.tensor.matmul(psum[:], w_sb[:, et, ft * P:(ft + 1) * P],
                                         xc[:, et, :],
                                         start=(et == 0), stop=(et == ET - 1))
                    if qk_copy == "scalar":
                        nc.scalar.copy(dst[:, ft, sc * QC:(sc + 1) * QC], psum[:])
                    else:
                        nc.vector.tensor_copy(dst[:, ft, sc * QC:(sc + 1) * QC], psum[:])

                def v_unit(wave, st):
                    si = st % KT_PER_QC
                    psum = ps_pp.tile([P, QC], F32, tag="pp")
                    for et in range(ET):
                        nc.tensor.matmul(psum[:], wave["xv"][:, et, si * P:(si + 1) * P],
                                         wv_sb[:, et, :],
                                         start=(et == 0), stop=(et == ET - 1))
                    nc.vector.tensor_copy(
                        vh_sb[:, st, :, 0:D],
                        psum[:].rearrange("p (h d) -> p h d", h=H))

                def p3_unit(jt, qc):
                    psum = ps_pp.tile([P, QC], F32, tag="pp")
                    for ft in range(FT):
                        nc.tensor.matmul(psum[:], wo_sb[:, ft, jt * P:(jt + 1) * P],
                                         xts_sb[:, ft, qc * QC:(qc + 1) * QC],
                                         start=(ft == 0), stop=(ft == FT - 1))
                    ot = otp.tile([P, QC], F32, tag="ot")
                    nc.vector.tensor_copy(ot[:], psum[:])
                    nc.sync.dma_start(
                        outT[jt * P:(jt + 1) * P, qc * QC:(qc + 1) * QC], ot[:])

                def head_block(h, qc):
                    ft, fo = h // 2, (h % 2) * D
                    xt_t = ps_xt.tile([D + 1, QC], F32, tag="xt")
                    pend = []

                    def flush_one():
                        k0, n, at_t = pend.pop(0)
                        for j in range(n):
                            kt = k0 + j
                            nc.tensor.matmul(xt_t[:], vh_sb[:, kt, h, :],
                                             at_t[:, j * QC:(j + 1) * QC],
                                             start=(kt == 0), stop=(kt == 4 * qc + 3))

                    for (k0, n, mcol) in _plan_groups(qc):
                        sc_t = ps_sc.tile([P, 3 * QC], F32, tag="sc")
                        for j in range(n):
                            kt = k0 + j
                            nc.tensor.matmul(
                                sc_t[:, j * QC:(j + 1) * QC],
                                kh_sb[fo:fo + D, ft, kt * P:(kt + 1) * P],
                                qh_sb[fo:fo + D, ft, qc * QC:(qc + 1) * QC],
                                start=True, stop=True)
                        w = n * QC
                        at_t = attnp.tile([P, 3 * QC] if n > 1 else [P, QC], BF16,
                                          tag=("at3" if n > 1 else "at1"))
                        nc.scalar.activation(at_t[:, 0:w], sc_t[:, 0:w],
                                             mybir.ActivationFunctionType.Exp,
                                             scale=0.125)
                        if mcol is not None:
                            nc.vector.tensor_mul(at_t[:, 0:w], at_t[:, 0:w],
                                                 dmask_sb[:, mcol:mcol + w])
                        pend.append((k0, n, at_t))
                        if len(pend) > lag:
                            flush_one()
                    while pend:
                        flush_one()
                    # normalize
                    recip = smalls.tile([1, QC], F32, tag="recip")
                    nc.vector.reciprocal(recip[:], xt_t[D:D + 1, :])
                    rb = smalls.tile([D, QC], F32, tag="rb")
                    nc.gpsimd.partition_broadcast(rb[:], recip[0:1, :])
                    nc.vector.tensor_mul(
                        xts_sb[fo:fo + D, ft, qc * QC:(qc + 1) * QC],
                        xt_t[0:D, :], rb[:])

                # ---- lead-in ----
                cur = dma_wave(0)
                qk_unit(cur, "q", 0, 0)
                qk_unit(cur, "k", 0, 0)
                for st in range(KT_PER_QC):
                    v_unit(cur, st)

                # ---- waves ----
                for qc in range(NQC):
                    nxt = dma_wave(qc + 1) if qc + 1 < NQC else None
                    proj_fill = ([(s, f) for f in range(FT) for s in ("q", "k")]
                                 if nxt else [])
                    v_fill = ([4 * (qc + 1) + i for i in range(KT_PER_QC)]
                              if nxt else [])
                    p3_fill = [(jt, qc - 1) for jt in range(ET)] if qc >= 1 else []
                    for h in range(H):
                        if qc == 0 and h >= 2 and h % 2 == 0:
                            qk_unit(cur, "q", h // 2, 0)
                            qk_unit(cur, "k", h // 2, 0)
                        head_block(h, qc)
                        if nxt:
                            s, f = proj_fill[h]
                            qk_unit(nxt, s, f, qc + 1)
                            if h % 2 == 1:
                                v_unit(nxt, v_fill[h // 2])
                        if p3_fill:
                            jt, qcp = p3_fill[h]
                            p3_unit(jt, qcp)
                    cur = nxt

                # ---- tail ----
                for jt in range(ET):
                    p3_unit(jt, NQC - 1)

            if niter is None:
                body()
            else:
                assert niter % unroll == 0
                with tc.For_i(0, niter // unroll, 1):
                    for _ in range(unroll):
                        body()

    nc.compile()
    return nc


def build_nc3(niter=None, lag=2, at_bufs3=4, pp_bufs=2, exact_recip=True,
              split_exp=False, serial_scores=False, norm_mode="wave",
              strip_mask=True, narrow_diag=True, qk_on_scalar=True):
    """Causal-only v3.  Per (qc, head-pair) block: at each kt step, two
    row-tiled concurrent K=64 scores matmuls (rows 0-63 / 64-127) write one
    2-bank PSUM pair-tile, a single FD=1024 exp converts both, a duplicated
    mask handles the diagonal, and two attn@V matmuls accumulate per-head
    xt.  Projection + output-projection units are interleaved as PE filler."""
    nc = bacc.Bacc("TRN2", target_bir_lowering=False, debug=False,
                   enable_asserts=True, num_devices=8)

    qT = nc.dram_tensor("qT", [E, S], BF16, kind="ExternalInput").ap()
    kT = nc.dram_tensor("kT", [E, S], BF16, kind="ExternalInput").ap()
    vT = nc.dram_tensor("vT", [E, S], BF16, kind="ExternalInput").ap()
    wqT = nc.dram_tensor("wqT", [E, F], BF16, kind="ExternalInput").ap()
    wkT = nc.dram_tensor("wkT", [E, F], BF16, kind="ExternalInput").ap()
    wvT = nc.dram_tensor("wvT", [E, F], BF16, kind="ExternalInput").ap()
    woT = nc.dram_tensor("woT", [F, E], BF16, kind="ExternalInput").ap()
    dmask2 = nc.dram_tensor("dmask2", [P, 8 * QC], BF16, kind="ExternalInput").ap()
    outT = nc.dram_tensor("outT", [E, S], F32, kind="ExternalOutput").ap()

    qT3 = qT.rearrange("(o p) s -> p o s", p=P)
    kT3 = kT.rearrange("(o p) s -> p o s", p=P)
    vT3 = vT.rearrange("(o p) s -> p o s", p=P)

    with tile.TileContext(nc) as tc:
        import contextlib
        with contextlib.ExitStack() as ctx:
            persist = ctx.enter_context(tc.tile_pool(name="persist", bufs=1))
            streams = ctx.enter_context(tc.tile_pool(name="streams", bufs=2))
            otp = ctx.enter_context(tc.tile_pool(name="otp", bufs=3))
            attnp = ctx.enter_context(tc.tile_pool(name="attnp", bufs=at_bufs3))
            smalls = ctx.enter_context(tc.tile_pool(name="smalls", bufs=3))
            xtcp = ctx.enter_context(tc.tile_pool(name="xtcp", bufs=10))
            ps_sc = ctx.enter_context(tc.tile_pool(name="ps_sc", bufs=2, space="PSUM"))
            ps_xt = ctx.enter_context(tc.tile_pool(name="ps_xt", bufs=2, space="PSUM"))
            ps_pp = ctx.enter_context(tc.tile_pool(name="ps_pp", bufs=pp_bufs, space="PSUM"))

            wq_sb = persist.tile([P, ET, F], BF16, tag="wq")
            wk_sb = persist.tile([P, ET, F], BF16, tag="wk")
            wv_sb = persist.tile([P, ET, F], BF16, tag="wv")
            wo_sb = persist.tile([P, FT, E], BF16, tag="wo")
            dm_sb = persist.tile([P, 8 * QC], BF16, tag="dmask2")
            nc.sync.dma_start(wq_sb[:], wqT.rearrange("(o p) f -> p o f", p=P))
            nc.sync.dma_start(wk_sb[:], wkT.rearrange("(o p) f -> p o f", p=P))
            nc.sync.dma_start(wv_sb[:], wvT.rearrange("(o p) f -> p o f", p=P))
            nc.sync.dma_start(wo_sb[:], woT.rearrange("(o p) e -> p o e", p=P))
            nc.sync.dma_start(dm_sb[:], dmask2[:])

            qh_sb = persist.tile([P, FT, S], BF16, tag="qh")
            kh_sb = persist.tile([P, FT, S], BF16, tag="kh")
            vh_sb = persist.tile([P, ST, H, D + 1], BF16, tag="vh")
            xts_sb = persist.tile([P, FT, S], BF16, tag="xts")

            def body():
                nc.vector.memset(vh_sb[:, :, :, D:D + 1], 1.0)

                def dma_wave(sc):
                    w = {}
                    for tag, src3 in (("xq", qT3), ("xk", kT3), ("xv", vT3)):
                        t = streams.tile([P, ET, QC], BF16, tag=tag)
                        nc.sync.dma_start(t[:], src3[:, :, sc * QC:(sc + 1) * QC])
                        w[tag] = t
                    return w

                def qk_unit(wave, which, ft, sc):
                    w_sb, dst, xc = ((wq_sb, qh_sb, wave["xq"]) if which == "q"
                                     else (wk_sb, kh_sb, wave["xk"]))
                    psum = ps_pp.tile([P, QC], F32, tag="pp")
                    for et in range(ET):
                        nc.tensor.matmul(psum[:], w_sb[:, et, ft * P:(ft + 1) * P],
                                         xc[:, et, :],
                                         start=(et == 0), stop=(et == ET - 1))
                    if qk_on_scalar:
                        nc.scalar.copy(dst[:, ft, sc * QC:(sc + 1) * QC], psum[:])
                    else:
                        nc.vector.tensor_copy(dst[:, ft, sc * QC:(sc + 1) * QC], psum[:])

                def v_unit(wave, st):
                    si = st % KT_PER_QC
                    psum = ps_pp.tile([P, QC], F32, tag="pp")
                    for et in range(ET):
                        nc.tensor.matmul(psum[:], wave["xv"][:, et, si * P:(si + 1) * P],
                                         wv_sb[:, et, :],
                                         start=(et == 0), stop=(et == ET - 1))
                    nc.vector.tensor_copy(
                        vh_sb[:, st, :, 0:D],
                        psum[:].rearrange("p (h d) -> p h d", h=H))

                def p3_unit(jt, qc):
                    psum = ps_pp.tile([P, QC], F32, tag="pp")
                    for ft in range(FT):
                        nc.tensor.matmul(psum[:], wo_sb[:, ft, jt * P:(jt + 1) * P],
                                         xts_sb[:, ft, qc * QC:(qc + 1) * QC],
                                         start=(ft == 0), stop=(ft == FT - 1))
                    ot = otp.tile([P, QC], F32, tag="ot")
                    nc.vector.tensor_copy(ot[:], psum[:])
                    nc.sync.dma_start(
                        outT[jt * P:(jt + 1) * P, qc * QC:(qc + 1) * QC], ot[:])

                def normalize(xt_t, h, qc):
                    ft, fo = h // 2, (h % 2) * D
                    recip = smalls.tile([1, QC], F32, tag="recip")
                    if exact_recip:
                        nc.vector.reciprocal(recip[:], xt_t[D:D + 1, :])
                    else:
                        nc.vector.reciprocal_approx_fast(recip[:], xt_t[D:D + 1, :])
                    rb = smalls.tile([D, QC], F32, tag="rb")
                    nc.gpsimd.partition_broadcast(rb[:], recip[0:1, :])
                    nc.vector.tensor_mul(
                        xts_sb[fo:fo + D, ft, qc * QC:(qc + 1) * QC],
                        xt_t[0:D, :], rb[:])

                def pair_block(p, qc, wave_norm):
                    h0, h1 = 2 * p, 2 * p + 1
                    ft = p
                    nkt = 4 * qc + 4
                    xt0 = ps_xt.tile([D + 1, QC], F32, tag="xt")
                    xt1 = ps_xt.tile([D + 1, QC], F32, tag="xt")
                    pend = []

                    def flush_one():
                        kt, at_t = pend.pop(0)
                        st, sp = (kt == 0), (kt == nkt - 1)
                        jd = kt - 4 * qc
                        o = jd * P if (narrow_diag and jd > 0) else 0
                        nc.tensor.matmul(xt0[:, o:QC], vh_sb[:, kt, h0, :],
                                         at_t[:, o:QC], start=st, stop=sp)
                        nc.tensor.matmul(xt1[:, o:QC], vh_sb[:, kt, h1, :],
                                         at_t[:, QC + o:2 * QC], start=st, stop=sp)

                    for kt in range(nkt):
                        sc_t = ps_sc.tile([P, 2 * QC], F32, tag="sc")
                        for fo, cs in ((0, slice(0, QC)), (D, slice(QC, 2 * QC))):
                            nc.tensor.matmul(
                                sc_t[:, cs],
                                kh_sb[fo:fo + D, ft, kt * P:(kt + 1) * P],
                                qh_sb[fo:fo + D, ft, qc * QC:(qc + 1) * QC],
                                start=True, stop=True)
                        at_t = attnp.tile([P, 2 * QC], BF16, tag="at")
                        if split_exp:
                            for cs in (slice(0, QC), slice(QC, 2 * QC)):
                                nc.scalar.activation(at_t[:, cs], sc_t[:, cs],
                                                     mybir.ActivationFunctionType.Exp,
                                                     scale=0.125)
                        else:
                            nc.scalar.activation(at_t[:], sc_t[:],
                                                 mybir.ActivationFunctionType.Exp,
                                                 scale=0.125)
                        jd = kt - 4 * qc
                        if jd >= 0:
                            if strip_mask:
                                for half in (0, QC):
                                    s0 = half + jd * P
                                    nc.vector.tensor_mul(
                                        at_t[:, s0:s0 + P], at_t[:, s0:s0 + P],
                                        dm_sb[:, 0:P])
                                    if not narrow_diag and jd > 0:
                                        nc.vector.memset(at_t[:, half:half + jd * P], 0.0)
                            else:
                                nc.vector.tensor_mul(
                                    at_t[:], at_t[:],
                                    dm_sb[:, jd * 2 * QC:(jd + 1) * 2 * QC])
                        pend.append((kt, at_t))
                        if len(pend) > lag:
                            flush_one()
                    while pend:
                        flush_one()
                    if norm_mode == "wave":
                        for h, xt_t in ((h0, xt0), (h1, xt1)):
                            xtc = xtcp.tile([D + 1, QC], BF16, tag="xtc")
                            nc.vector.tensor_copy(xtc[:], xt_t[:])
                            wave_norm.append((h, xtc))
                    else:
                        normalize(xt0, h0, qc)
                        normalize(xt1, h1, qc)

                def wave_normalize(wave_norm, qc):
                    # Pack denominators 4-per-tile at partitions {0,32,64,96}
                    # (cross-partition moves must go through DMA) so one
                    # FD-bound reciprocal covers 4 heads.
                    for half in range(2):
                        quad = wave_norm[4 * half:4 * half + 4]
                        dnb = smalls.tile([97, QC], BF16, tag="dnb")
                        nc.vector.memset(dnb[:], 1.0)
                        for j, (h, xtc) in enumerate(quad):
                            nc.sync.dma_start(dnb[32 * j:32 * j + 1, :],
                                              xtc[D:D + 1, :])
                        dn = smalls.tile([97, QC], F32, tag="dn")
                        nc.vector.tensor_copy(dn[:], dnb[:])
                        rc = smalls.tile([97, QC], F32, tag="rc")
                        nc.vector.reciprocal(rc[:], dn[:])
                        for j, (h, xtc) in enumerate(quad):
                            ft, fo = h // 2, (h % 2) * D
                            rrow = smalls.tile([1, QC], F32, tag="rrow")
                            nc.sync.dma_start(rrow[:], rc[32 * j:32 * j + 1, :])
                            rb = smalls.tile([D, QC], F32, tag="rb")
                            nc.gpsimd.partition_broadcast(rb[:], rrow[0:1, :])
                            nc.vector.tensor_mul(
                                xts_sb[fo:fo + D, ft, qc * QC:(qc + 1) * QC],
                                xtc[0:D, :], rb[:])

                # ---- lead-in ----
                cur = dma_wave(0)
                qk_unit(cur, "q", 0, 0)
                qk_unit(cur, "k", 0, 0)
                for st in range(KT_PER_QC):
                    v_unit(cur, st)

                # ---- waves ----
                for qc in range(NQC):
                    nxt = dma_wave(qc + 1) if qc + 1 < NQC else None
                    wave_norm = []
                    for p in range(FT):
                        if qc == 0 and p >= 1:
                            qk_unit(cur, "q", p, 0)
                            qk_unit(cur, "k", p, 0)
                        pair_block(p, qc, wave_norm)
                        if nxt:
                            qk_unit(nxt, "q", p, qc + 1)
                            qk_unit(nxt, "k", p, qc + 1)
                            v_unit(nxt, 4 * (qc + 1) + p)
                        if qc >= 1:
                            p3_unit(2 * p, qc - 1)
                            p3_unit(2 * p + 1, qc - 1)
                    if norm_mode == "wave":
                        wave_normalize(wave_norm, qc)
                    cur = nxt

                # ---- tail ----
                for jt in range(ET):
                    p3_unit(jt, NQC - 1)

            if niter is None:
                body()
            else:
                with tc.For_i(0, niter, 1):
                    body()

    nc.compile()
    return nc


def _host_prep(q, k, v, mask, w_q, w_k, w_v, w_o):
    """Shard + transpose inputs on the host.  Returns (in_maps, causal)."""
    tril = np.tril(np.ones((S, S), dtype=mask.dtype))
    causal = all(np.array_equal(np.asarray(mask[b, 0]), tril) for b in range(B))

    stair = (np.arange(2 * QC)[None, :] >= (np.arange(P)[:, None] + QC))
    stair = stair.astype(NPBF16)

    # v2 diag mask: 4 concatenated [P, QC] tiles, tile jd valid iff q >= p + 128*jd
    dmask = np.concatenate(
        [(np.arange(QC)[None, :] >= (np.arange(P)[:, None] + P * jd))
         for jd in range(4)], axis=1).astype(NPBF16)
    # v3 pair mask: same, but each jd tile duplicated side-by-side for the
    # two heads of a pair sharing one [P, 2*QC] at-tile
    dmask2 = np.concatenate(
        [np.tile((np.arange(QC)[None, :] >= (np.arange(P)[:, None] + P * jd)), (1, 2))
         for jd in range(4)], axis=1).astype(NPBF16)

    w_q = np.asarray(w_q, dtype=np.float32)
    w_k = np.asarray(w_k, dtype=np.float32)
    w_v = np.asarray(w_v, dtype=np.float32)
    w_o = np.asarray(w_o, dtype=np.float32)

    in_maps = []
    for core in range(8):
        b, g = divmod(core, 2)
        rows = slice(g * F, (g + 1) * F)
        m = {
            "qT": np.ascontiguousarray(np.asarray(q[b], np.float32).T).astype(NPBF16),
            "kT": np.ascontiguousarray(np.asarray(k[b], np.float32).T).astype(NPBF16),
            "vT": np.ascontiguousarray(np.asarray(v[b], np.float32).T).astype(NPBF16),
            "wqT": np.ascontiguousarray(w_q[rows, :].T).astype(NPBF16),
            "wkT": np.ascontiguousarray(w_k[rows, :].T).astype(NPBF16),
            "wvT": np.ascontiguousarray(w_v[rows, :].T).astype(NPBF16),
            "woT": np.ascontiguousarray(w_o[:, rows].T).astype(NPBF16),
            "stair": stair,
            "dmask": dmask,
            "dmask2": dmask2,
        }
        if not causal:
            m["maskT"] = np.ascontiguousarray(
                np.asarray(mask[b, 0], np.float32).T).astype(NPBF16)
        in_maps.append(m)
    return in_maps, causal


_NC_CACHE: dict = {}


def kernel(q, k, v, mask, w_q, w_k, w_v, w_o):
    in_maps, causal = _host_prep(q, k, v, mask, w_q, w_k, w_v, w_o)
    nc = _NC_CACHE.get(causal)
    if nc is None:
        nc = build_nc3() if causal else build_nc(causal)
        _NC_CACHE[causal] = nc
    res = bass_utils.run_bass_kernel_spmd(nc, in_maps, core_ids=list(range(8)))
    out = np.empty((B, S, E), dtype=np.float32)
    for b in range(B):
        out[b] = (res.results[2 * b]["outT"] + res.results[2 * b + 1]["outT"]).T
    return out



# revision 27
# speedup vs baseline: 1.4455x; 1.0353x over previous
"""Multi-head attention block kernel for Trainium2, sharded over 8 NeuronCores.

Sharding: batch (4) x head-group (2 groups of 8 heads) -> 8 cores.
Each core computes, for one batch b and one half of the heads:
  qh/kh/vh projections (columns of w_q/w_k/w_v for its heads),
  causal attention for its 8 heads, and a partial output projection
  (rows of w_o^T for its heads).  Host sums the two partial outputs per
  batch and transposes back.

On-chip layout is feature-major ("transposed"): activations live as
[feature, seq] so every matmul contraction dim is on partitions and no
on-chip transposes are needed.  Host pre-transposes q/k/v and the
weight slices, and post-transposes the output.

Matmuls run in bf16 (fp32 matmul is 4x slower on TRN2); accumulation is
fp32 in PSUM.  Softmax denominators come for free from an extra ones
column appended to each V tile (row 64 of the attn@V accumulator is the
sum of exp scores).
"""

import sys

sys.path.insert(0, "/opt/trn_rl_repo")

import numpy as np
import ml_dtypes

import concourse.bacc as bacc
import concourse.mybir as mybir
import concourse.tile as tile
from concourse import bass_utils

B = 4
S = 2048
E = 1024
HEADS = 16
D = 64
H = 8            # heads per core
F = H * D        # 512 local head features
P = 128
ET = E // P      # 8 e-tiles
FT = F // P      # 4 f-tiles
ST = S // P      # 16 s-tiles
QC = 512         # q-chunk width
NQC = S // QC    # 4 q-chunks
KT_PER_QC = QC // P  # 4 k-tiles per q-chunk

BF16 = mybir.dt.bfloat16
F32 = mybir.dt.float32
NPBF16 = ml_dtypes.bfloat16


def build_nc(causal: bool, niter: int | None = None, phases=(1, 2, 3), no_norm=False, no_exp=False,
             p1_wide=4, p3_wide=4, xtlag=2, sc_bufs=0, ps_bufs=4, at_bufs=12, old_p2=False):
    """Build the per-core Bass program.  If niter is given, wrap the body in a
    For_i timing loop (used by test.py to measure HW time)."""
    nc = bacc.Bacc("TRN2", target_bir_lowering=False, debug=False,
                   enable_asserts=True, num_devices=8)

    qT = nc.dram_tensor("qT", [E, S], BF16, kind="ExternalInput").ap()
    kT = nc.dram_tensor("kT", [E, S], BF16, kind="ExternalInput").ap()
    vT = nc.dram_tensor("vT", [E, S], BF16, kind="ExternalInput").ap()
    wqT = nc.dram_tensor("wqT", [E, F], BF16, kind="ExternalInput").ap()
    wkT = nc.dram_tensor("wkT", [E, F], BF16, kind="ExternalInput").ap()
    wvT = nc.dram_tensor("wvT", [E, F], BF16, kind="ExternalInput").ap()
    woT = nc.dram_tensor("woT", [F, E], BF16, kind="ExternalInput").ap()
    stair = nc.dram_tensor("stair", [P, 2 * QC], BF16, kind="ExternalInput").ap()
    if not causal:
        maskT = nc.dram_tensor("maskT", [S, S], BF16, kind="ExternalInput").ap()
    outT = nc.dram_tensor("outT", [E, S], F32, kind="ExternalOutput").ap()

    qT3 = qT.rearrange("(o p) s -> p o s", p=P)
    kT3 = kT.rearrange("(o p) s -> p o s", p=P)
    vT3 = vT.rearrange("(o p) s -> p o s", p=P)
    if not causal:
        maskT3 = maskT.rearrange("(o p) s -> p o s", p=P)

    with tile.TileContext(nc) as tc:
        import contextlib
        with contextlib.ExitStack() as ctx:
            persist = ctx.enter_context(tc.tile_pool(name="persist", bufs=1))
            streams = ctx.enter_context(tc.tile_pool(name="streams", bufs=6))
            attnp = ctx.enter_context(tc.tile_pool(name="attnp", bufs=at_bufs))
            smalls = ctx.enter_context(tc.tile_pool(name="smalls", bufs=3))
            ps1 = ctx.enter_context(tc.tile_pool(name="ps1", bufs=ps_bufs, space="PSUM"))
            if sc_bufs:
                ps_sc = ctx.enter_context(tc.tile_pool(name="ps_sc", bufs=sc_bufs, space="PSUM"))
            ps_xt = ctx.enter_context(tc.tile_pool(name="ps_xt", bufs=1, space="PSUM"))
            def sc_tile():
                if sc_bufs:
                    return ps_sc.tile([P, QC], F32, tag="sc", name="scp")
                return ps1.tile([P, QC], F32, tag="ps", name="scp")

            # Weights + constants: loaded once, outside the timing loop.
            wq_sb = persist.tile([P, ET, F], BF16, tag="wq")
            wk_sb = persist.tile([P, ET, F], BF16, tag="wk")
            wv_sb = persist.tile([P, ET, F], BF16, tag="wv")
            wo_sb = persist.tile([P, FT, E], BF16, tag="wo")
            stair_sb = persist.tile([P, 2 * QC], BF16, tag="stair")
            nc.sync.dma_start(wq_sb[:], wqT.rearrange("(o p) f -> p o f", p=P))
            nc.sync.dma_start(wk_sb[:], wkT.rearrange("(o p) f -> p o f", p=P))
            nc.sync.dma_start(wv_sb[:], wvT.rearrange("(o p) f -> p o f", p=P))
            nc.sync.dma_start(wo_sb[:], woT.rearrange("(o p) e -> p o e", p=P))
            nc.sync.dma_start(stair_sb[:], stair[:])

            # Persistent activations (bf16): projections and attention outputs.
            qh_sb = persist.tile([P, FT, S], BF16, tag="qh")    # [f, ft, s]
            kh_sb = persist.tile([P, FT, S], BF16, tag="kh")
            vh_sb = persist.tile([P, ST, H, D + 1], BF16, tag="vh")  # ones col at d=64
            xts_sb = persist.tile([P, FT, S], BF16, tag="xts")

            def body():
                run1 = 1 in phases
                run2 = 2 in phases
                run3 = 3 in phases
                if not run1:
                    nc.vector.memset(qh_sb[:, :, 0:1], 0.5)
                    nc.vector.memset(kh_sb[:, :, 0:1], 0.5)
                    nc.vector.memset(vh_sb[:, :, :, 0:1], 0.5)
                if not run2 and run3:
                    nc.vector.memset(xts_sb[:, :, 0:1], 0.5)
                # ---- Phase 1a: q/k projections -> qh/kh (feature-major) ----
                # Weight-stationary: for each (ft, et) weight tile, stream all
                # 4 s-chunks into 4 accumulating PSUMs so LDWEIGHTS happens
                # once per 4 matmuls.
                for src3, w_sb, dst in ((qT3, wq_sb, qh_sb), (kT3, wk_sb, kh_sb)) if run1 else ():
                    xcs = []
                    for sc in range(NQC):
                        xc = streams.tile([P, ET, QC], BF16, tag="xc")
                        nc.sync.dma_start(xc[:], src3[:, :, sc * QC:(sc + 1) * QC])
                        xcs.append(xc)
                    for ft in range(FT):
                        if p1_wide > 1:
                            for g0 in range(0, NQC, p1_wide):
                                gs = list(range(g0, min(NQC, g0 + p1_wide)))
                                psums = [ps1.tile([P, QC], F32, tag="ps", name=f"pp{sc}")
                                         for sc in gs]
                                for et in range(ET):
                                    for i, sc in enumerate(gs):
                                        nc.tensor.matmul(
                                            psums[i][:],
                                            w_sb[:, et, ft * P:(ft + 1) * P],
                                            xcs[sc][:, et, :],
                                            start=(et == 0), stop=(et == ET - 1))
                                for i, sc in enumerate(gs):
                                    nc.vector.tensor_copy(
                                        dst[:, ft, sc * QC:(sc + 1) * QC], psums[i][:])
                        else:
                            for sc in range(NQC):
                                psum = ps1.tile([P, QC], F32, tag="ps", name="pp")
                                for et in range(ET):
                                    nc.tensor.matmul(
                                        psum[:],
                                        w_sb[:, et, ft * P:(ft + 1) * P],
                                        xcs[sc][:, et, :],
                                        start=(et == 0), stop=(et == ET - 1))
                                nc.vector.tensor_copy(
                                    dst[:, ft, sc * QC:(sc + 1) * QC], psum[:])

                # ---- Phase 1b: v projection -> vh (seq-major) + ones column ----
                nc.vector.memset(vh_sb[:, :, :, D:D + 1], 1.0)
                for sc in range(NQC) if run1 else ():
                    xc = streams.tile([P, ET, QC], BF16, tag="xc")
                    nc.sync.dma_start(xc[:], vT3[:, :, sc * QC:(sc + 1) * QC])
                    for si in range(KT_PER_QC):
                        st = sc * KT_PER_QC + si
                        psum = ps1.tile([P, QC], F32, tag="ps")
                        for et in range(ET):
                            nc.tensor.matmul(
                                psum[:],
                                xc[:, et, si * P:(si + 1) * P],
                                wv_sb[:, et, :],
                                start=(et == 0), stop=(et == ET - 1))
                        nc.vector.tensor_copy(
                            vh_sb[:, st, :, 0:D],
                            psum[:].rearrange("p (h d) -> p h d", h=H))

                # ---- Phase 2: attention ----
                # Causal path: kt-outer so the stationary operands (K tile for
                # scores, V tile for attn@V) are each loaded once per (h, kt)
                # and streamed over all valid q-chunks (LDWEIGHTS amortization;
                # weight switches cost ~250ns on PE).  Needs one xt accumulator
                # per q-chunk (4 PSUM banks).
                def normalize(xt_psum, h, qc):
                    ft, fo = h // 2, (h % 2) * D
                    if no_norm:
                        nc.vector.tensor_copy(
                            xts_sb[fo:fo + D, ft, qc * QC:(qc + 1) * QC],
                            xt_psum[0:D, :])
                    else:
                        recip = smalls.tile([1, QC], F32, tag="recip")
                        nc.vector.reciprocal(recip[:], xt_psum[D:D + 1, :])
                        rb = smalls.tile([D, QC], F32, tag="rb")
                        nc.gpsimd.partition_broadcast(rb[:], recip[0:1, :])
                        nc.vector.tensor_mul(
                            xts_sb[fo:fo + D, ft, qc * QC:(qc + 1) * QC],
                            xt_psum[0:D, :], rb[:])

                def emit_exp(at, sc_psum, kt, qc, mc):
                    if no_exp:
                        nc.vector.tensor_copy(at[:], sc_psum[:])
                    else:
                        nc.scalar.activation(at[:], sc_psum[:],
                                             mybir.ActivationFunctionType.Exp,
                                             scale=0.125)
                    if causal:
                        if kt // KT_PER_QC == qc:
                            off = kt * P - qc * QC
                            nc.vector.tensor_mul(
                                at[:], at[:], stair_sb[:, QC - off:2 * QC - off])
                    else:
                        nc.vector.tensor_mul(at[:], at[:], mc[:, kt, :])

                if run2 and causal and not old_p2:
                    for h in range(H):
                        ft, fo = h // 2, (h % 2) * D
                        xt_psums = [ps_xt.tile([D + 1, QC], F32, tag=f"xt{qc}", name=f"xt{qc}")
                                    for qc in range(NQC)]
                        pend = []   # [(kt, qc, at)] generations awaiting attn@V

                        def flush_xt(gen):
                            for kt, qc, at in gen:
                                nc.tensor.matmul(
                                    xt_psums[qc][:],
                                    vh_sb[:, kt, h, :],
                                    at[:],
                                    start=(kt == 0),
                                    stop=(kt == (qc + 1) * KT_PER_QC - 1))

                        XTLAG = xtlag
                        for kt in range(ST):
                            qcs = [qc for qc in range(NQC)
                                   if kt < (qc + 1) * KT_PER_QC]
                            nxt = []
                            for qc in qcs:
                                sc_psum = sc_tile()
                                nc.tensor.matmul(
                                    sc_psum[:],
                                    kh_sb[fo:fo + D, ft, kt * P:(kt + 1) * P],
                                    qh_sb[fo:fo + D, ft, qc * QC:(qc + 1) * QC],
                                    start=True, stop=True)
                                at = attnp.tile([P, QC], BF16, tag="at")
                                emit_exp(at, sc_psum, kt, qc, None)
                                nxt.append((kt, qc, at))
                            pend.append(nxt)
                            if len(pend) > XTLAG:
                                flush_xt(pend.pop(0))
                        for gen in pend:
                            flush_xt(gen)
                        for qc in range(NQC):
                            normalize(xt_psums[qc], h, qc)

                elif run2:
                    # general-mask path: qc-outer, mask tiles streamed per qc.
                    # (also used as the old_p2 comparison structure for causal)
                    for qc in range(NQC):
                        if causal:
                            mc = None
                            ktm = (qc + 1) * KT_PER_QC
                        else:
                            mc = streams.tile([P, ST, QC], BF16, tag="mc")
                            nc.sync.dma_start(mc[:], maskT3[:, :, qc * QC:(qc + 1) * QC])
                            ktm = ST
                        for h in range(H):
                            ft, fo = h // 2, (h % 2) * D
                            xt_psum = ps_xt.tile([D + 1, QC], F32, tag="xt0")
                            at_tiles = [None] * ktm

                            def emit_sc(kt):
                                sc_psum = sc_tile()
                                nc.tensor.matmul(
                                    sc_psum[:],
                                    kh_sb[fo:fo + D, ft, kt * P:(kt + 1) * P],
                                    qh_sb[fo:fo + D, ft, qc * QC:(qc + 1) * QC],
                                    start=True, stop=True)
                                at = attnp.tile([P, QC], BF16, tag="at")
                                emit_exp(at, sc_psum, kt, qc, mc)
                                at_tiles[kt] = at

                            def emit_xt(kt):
                                nc.tensor.matmul(
                                    xt_psum[:],
                                    vh_sb[:, kt, h, :],
                                    at_tiles[kt][:],
                                    start=(kt == 0), stop=(kt == ktm - 1))

                            PIPE = 2
                            for kt in range(ktm):
                                emit_sc(kt)
                                if kt >= PIPE:
                                    emit_xt(kt - PIPE)
                            for kt in range(max(0, ktm - PIPE), ktm):
                                emit_xt(kt)
                            normalize(xt_psum, h, qc)

                # ---- Phase 3: output projection (partial over local heads) ----
                for jt in range(ET) if run3 else ():
                    if p3_wide > 1:
                        psums = [ps1.tile([P, QC], F32, tag="ps", name=f"po{qc}")
                                 for qc in range(NQC)]
                        for ft in range(FT):
                            for qc in range(NQC):
                                nc.tensor.matmul(
                                    psums[qc][:],
                                    wo_sb[:, ft, jt * P:(jt + 1) * P],
                                    xts_sb[:, ft, qc * QC:(qc + 1) * QC],
                                    start=(ft == 0), stop=(ft == FT - 1))
                        for qc in range(NQC):
                            ot = streams.tile([P, QC], F32, tag="ot")
                            nc.vector.tensor_copy(ot[:], psums[qc][:])
                            nc.sync.dma_start(
                                outT[jt * P:(jt + 1) * P, qc * QC:(qc + 1) * QC],
                                ot[:])
                    else:
                        for qc in range(NQC):
                            psum = ps1.tile([P, QC], F32, tag="ps", name="po")
                            for ft in range(FT):
                                nc.tensor.matmul(
                                    psum[:],
                                    wo_sb[:, ft, jt * P:(jt + 1) * P],
                                    xts_sb[:, ft, qc * QC:(qc + 1) * QC],
                                    start=(ft == 0), stop=(ft == FT - 1))
                            ot = streams.tile([P, QC], F32, tag="ot")
                            nc.vector.tensor_copy(ot[:], psum[:])
                            nc.sync.dma_start(
                                outT[jt * P:(jt + 1) * P, qc * QC:(qc + 1) * QC],
                                ot[:])

            if niter is None:
                body()
            else:
                with tc.For_i(0, niter, 1):
                    body()

    nc.compile()
    return nc


def _plan_groups(qc):
    """kt-tile groups for one (h, qc) block: non-diag groups of <=3 (no mask),
    then the 4 diagonal tiles as [3, 1] with fixed mask-table slices."""
    nd = 4 * qc
    groups = []
    k0 = 0
    while k0 < nd:
        n = min(3, nd - k0)
        groups.append((k0, n, None))
        k0 += n
    groups.append((nd, 3, 0))            # diag tiles jd=0..2 -> dmask cols [0, 1536)
    groups.append((nd + 3, 1, 3 * QC))   # diag tile jd=3 -> dmask cols [1536, 2048)
    return groups


def build_nc2(niter=None, lag=1, at3_bufs=3, qk_copy="scalar", unroll=1):
    """Causal-only v2: qc-major waves, batched exp over 3-bank PSUM groups,
    proj/out-proj units interleaved into the attention stream as PE filler."""
    nc = bacc.Bacc("TRN2", target_bir_lowering=False, debug=False,
                   enable_asserts=True, num_devices=8)

    qT = nc.dram_tensor("qT", [E, S], BF16, kind="ExternalInput").ap()
    kT = nc.dram_tensor("kT", [E, S], BF16, kind="ExternalInput").ap()
    vT = nc.dram_tensor("vT", [E, S], BF16, kind="ExternalInput").ap()
    wqT = nc.dram_tensor("wqT", [E, F], BF16, kind="ExternalInput").ap()
    wkT = nc.dram_tensor("wkT", [E, F], BF16, kind="ExternalInput").ap()
    wvT = nc.dram_tensor("wvT", [E, F], BF16, kind="ExternalInput").ap()
    woT = nc.dram_tensor("woT", [F, E], BF16, kind="ExternalInput").ap()
    dmask = nc.dram_tensor("dmask", [P, 4 * QC], BF16, kind="ExternalInput").ap()
    outT = nc.dram_tensor("outT", [E, S], F32, kind="ExternalOutput").ap()

    qT3 = qT.rearrange("(o p) s -> p o s", p=P)
    kT3 = kT.rearrange("(o p) s -> p o s", p=P)
    vT3 = vT.rearrange("(o p) s -> p o s", p=P)

    with tile.TileContext(nc) as tc:
        import contextlib
        with contextlib.ExitStack() as ctx:
            persist = ctx.enter_context(tc.tile_pool(name="persist", bufs=1))
            streams = ctx.enter_context(tc.tile_pool(name="streams", bufs=2))
            otp = ctx.enter_context(tc.tile_pool(name="otp", bufs=3))
            attnp = ctx.enter_context(tc.tile_pool(name="attnp", bufs=at3_bufs))
            smalls = ctx.enter_context(tc.tile_pool(name="smalls", bufs=3))
            ps_sc = ctx.enter_context(tc.tile_pool(name="ps_sc", bufs=2, space="PSUM"))
            ps_xt = ctx.enter_context(tc.tile_pool(name="ps_xt", bufs=1, space="PSUM"))
            ps_pp = ctx.enter_context(tc.tile_pool(name="ps_pp", bufs=1, space="PSUM"))

            wq_sb = persist.tile([P, ET, F], BF16, tag="wq")
            wk_sb = persist.tile([P, ET, F], BF16, tag="wk")
            wv_sb = persist.tile([P, ET, F], BF16, tag="wv")
            wo_sb = persist.tile([P, FT, E], BF16, tag="wo")
            dmask_sb = persist.tile([P, 4 * QC], BF16, tag="dmask")
            nc.sync.dma_start(wq_sb[:], wqT.rearrange("(o p) f -> p o f", p=P))
            nc.sync.dma_start(wk_sb[:], wkT.rearrange("(o p) f -> p o f", p=P))
            nc.sync.dma_start(wv_sb[:], wvT.rearrange("(o p) f -> p o f", p=P))
            nc.sync.dma_start(wo_sb[:], woT.rearrange("(o p) e -> p o e", p=P))
            nc.sync.dma_start(dmask_sb[:], dmask[:])

            qh_sb = persist.tile([P, FT, S], BF16, tag="qh")
            kh_sb = persist.tile([P, FT, S], BF16, tag="kh")
            vh_sb = persist.tile([P, ST, H, D + 1], BF16, tag="vh")
            xts_sb = persist.tile([P, FT, S], BF16, tag="xts")

            def body():
                nc.vector.memset(vh_sb[:, :, :, D:D + 1], 1.0)

                def dma_wave(sc):
                    w = {}
                    for tag, src3 in (("xq", qT3), ("xk", kT3), ("xv", vT3)):
                        t = streams.tile([P, ET, QC], BF16, tag=tag)
                        nc.sync.dma_start(t[:], src3[:, :, sc * QC:(sc + 1) * QC])
                        w[tag] = t
                    return w

                def qk_unit(wave, which, ft, sc):
                    w_sb, dst, xc = ((wq_sb, qh_sb, wave["xq"]) if which == "q"
                                     else (wk_sb, kh_sb, wave["xk"]))
                    psum = ps_pp.tile([P, QC], F32, tag="pp")
                    for et in range(ET):
                        nc.tensor.matmul(psum[:], w_sb[:, et, ft * P:(ft + 1) * P],
                                         xc[:, et, :],
                                         start=(et == 0), stop=(et == ET - 1))
                    if qk_copy == "scalar":
                        nc.scalar.copy(dst[:, ft, sc * QC:(sc + 1) * QC], psum[:])
                    else:
                        nc.vector.tensor_copy(dst[:, ft, sc * QC:(sc + 1) * QC], psum[:])

                def v_unit(wave, st):
                    si = st % KT_PER_QC
                    psum = ps_pp.tile([P, QC], F32, tag="pp")
                    for et in range(ET):
                        nc.tensor.matmul(psum[:], wave["xv"][:, et, si * P:(si + 1) * P],
                                         wv_sb[:, et, :],
                                         start=(et == 0), stop=(et == ET - 1))
                    nc.vector.tensor_copy(
                        vh_sb[:, st, :, 0:D],
                        psum[:].rearrange("p (h d) -> p h d", h=H))

                def p3_unit(jt, qc):
                    psum = ps_pp.tile([P, QC], F32, tag="pp")
                    for ft in range(FT):
                        nc.tensor.matmul(psum[:], wo_sb[:, ft, jt * P:(jt + 1) * P],
                                         xts_sb[:, ft, qc * QC:(qc + 1) * QC],
                                         start=(ft == 0), stop=(ft == FT - 1))
                    ot = otp.tile([P, QC], F32, tag="ot")
                    nc.vector.tensor_copy(ot[:], psum[:])
                    nc.sync.dma_start(
                        outT[jt * P:(jt + 1) * P, qc * QC:(qc + 1) * QC], ot[:])

                def head_block(h, qc):
                    ft, fo = h // 2, (h % 2) * D
                    xt_t = ps_xt.tile([D + 1, QC], F32, tag="xt")
                    pend = []

                    def flush_one():
                        k0, n, at_t = pend.pop(0)
                        for j in range(n):
                            kt = k0 + j
                            nc.tensor.matmul(xt_t[:], vh_sb[:, kt, h, :],
                                             at_t[:, j * QC:(j + 1) * QC],
                                             start=(kt == 0), stop=(kt == 4 * qc + 3))

                    for (k0, n, mcol) in _plan_groups(qc):
                        sc_t = ps_sc.tile([P, 3 * QC], F32, tag="sc")
                        for j in range(n):
                            kt = k0 + j
                            nc.tensor.matmul(
                                sc_t[:, j * QC:(j + 1) * QC],
                                kh_sb[fo:fo + D, ft, kt * P:(kt + 1) * P],
                                qh_sb[fo:fo + D, ft, qc * QC:(qc + 1) * QC],
                                start=True, stop=True)
                        w = n * QC
                        at_t = attnp.tile([P, 3 * QC] if n > 1 else [P, QC], BF16,
                                          tag=("at3" if n > 1 else "at1"))
                        nc.scalar.activation(at_t[:, 0:w], sc_t[:, 0:w],
                                             mybir.ActivationFunctionType.Exp,
                                             scale=0.125)
                        if mcol is not None:
                            nc.vector.tensor_mul(at_t[:, 0:w], at_t[:, 0:w],
                                                 dmask_sb[:, mcol:mcol + w])
                        pend.append((k0, n, at_t))
                        if len(pend) > lag:
                            flush_one()
                    while pend:
                        flush_one()
                    # normalize
                    recip = smalls.tile([1, QC], F32, tag="recip")
                    nc.vector.reciprocal(recip[:], xt_t[D:D + 1, :])
                    rb = smalls.tile([D, QC], F32, tag="rb")
                    nc.gpsimd.partition_broadcast(rb[:], recip[0:1, :])
                    nc.vector.tensor_mul(
                        xts_sb[fo:fo + D, ft, qc * QC:(qc + 1) * QC],
                        xt_t[0:D, :], rb[:])

                # ---- lead-in ----
                cur = dma_wave(0)
                qk_unit(cur, "q", 0, 0)
                qk_unit(cur, "k", 0, 0)
                for st in range(KT_PER_QC):
                    v_unit(cur, st)

                # ---- waves ----
                for qc in range(NQC):
                    nxt = dma_wave(qc + 1) if qc + 1 < NQC else None
                    proj_fill = ([(s, f) for f in range(FT) for s in ("q", "k")]
                                 if nxt else [])
                    v_fill = ([4 * (qc + 1) + i for i in range(KT_PER_QC)]
                              if nxt else [])
                    p3_fill = [(jt, qc - 1) for jt in range(ET)] if qc >= 1 else []
                    for h in range(H):
                        if qc == 0 and h >= 2 and h % 2 == 0:
                            qk_unit(cur, "q", h // 2, 0)
                            qk_unit(cur, "k", h // 2, 0)
                        head_block(h, qc)
                        if nxt:
                            s, f = proj_fill[h]
                            qk_unit(nxt, s, f, qc + 1)
                            if h % 2 == 1:
                                v_unit(nxt, v_fill[h // 2])
                        if p3_fill:
                            jt, qcp = p3_fill[h]
                            p3_unit(jt, qcp)
                    cur = nxt

                # ---- tail ----
                for jt in range(ET):
                    p3_unit(jt, NQC - 1)

            if niter is None:
                body()
            else:
                assert niter % unroll == 0
                with tc.For_i(0, niter // unroll, 1):
                    for _ in range(unroll):
                        body()

    nc.compile()
    return nc


def build_nc3(niter=None, lag=2, at_bufs3=4, pp_bufs=2, exact_recip=True,
              split_exp=False, serial_scores=False, norm_mode="wave",
              strip_mask=True, narrow_diag=True, qk_on_scalar=True):
    """Causal-only v3.  Per (qc, head-pair) block: at each kt step, two
    row-tiled concurrent K=64 scores matmuls (rows 0-63 / 64-127) write one
    2-bank PSUM pair-tile, a single FD=1024 exp converts both, a duplicated
    mask handles the diagonal, and two attn@V matmuls accumulate per-head
    xt.  Projection + output-projection units are interleaved as PE filler."""
    nc = bacc.Bacc("TRN2", target_bir_lowering=False, debug=False,
                   enable_asserts=True, num_devices=8)

    qT = nc.dram_tensor("qT", [E, S], BF16, kind="ExternalInput").ap()
    kT = nc.dram_tensor("kT", [E, S], BF16, kind="ExternalInput").ap()
    vT = nc.dram_tensor("vT", [E, S], BF16, kind="ExternalInput").ap()
    wqT = nc.dram_tensor("wqT", [E, F], BF16, kind="ExternalInput").ap()
    wkT = nc.dram_tensor("wkT", [E, F], BF16, kind="ExternalInput").ap()
    wvT = nc.dram_tensor("wvT", [E, F], BF16, kind="ExternalInput").ap()
    woT = nc.dram_tensor("woT", [F, E], BF16, kind="ExternalInput").ap()
    dmask2 = nc.dram_tensor("dmask2", [P, 8 * QC], BF16, kind="ExternalInput").ap()
    outT = nc.dram_tensor("outT", [E, S], F32, kind="ExternalOutput").ap()

    qT3 = qT.rearrange("(o p) s -> p o s", p=P)
    kT3 = kT.rearrange("(o p) s -> p o s", p=P)
    vT3 = vT.rearrange("(o p) s -> p o s", p=P)

    with tile.TileContext(nc) as tc:
        import contextlib
        with contextlib.ExitStack() as ctx:
            persist = ctx.enter_context(tc.tile_pool(name="persist", bufs=1))
            streams = ctx.enter_context(tc.tile_pool(name="streams", bufs=2))
            otp = ctx.enter_context(tc.tile_pool(name="otp", bufs=3))
            attnp = ctx.enter_context(tc.tile_pool(name="attnp", bufs=at_bufs3))
            smalls = ctx.enter_context(tc.tile_pool(name="smalls", bufs=3))
            xtcp = ctx.enter_context(tc.tile_pool(name="xtcp", bufs=10))
            ps_sc = ctx.enter_context(tc.tile_pool(name="ps_sc", bufs=2, space="PSUM"))
            ps_xt = ctx.enter_context(tc.tile_pool(name="ps_xt", bufs=2, space="PSUM"))
            ps_pp = ctx.enter_context(tc.tile_pool(name="ps_pp", bufs=pp_bufs, space="PSUM"))

            wq_sb = persist.tile([P, ET, F], BF16, tag="wq")
            wk_sb = persist.tile([P, ET, F], BF16, tag="wk")
            wv_sb = persist.tile([P, ET, F], BF16, tag="wv")
            wo_sb = persist.tile([P, FT, E], BF16, tag="wo")
            dm_sb = persist.tile([P, 8 * QC], BF16, tag="dmask2")
            nc.sync.dma_start(wq_sb[:], wqT.rearrange("(o p) f -> p o f", p=P))
            nc.sync.dma_start(wk_sb[:], wkT.rearrange("(o p) f -> p o f", p=P))
            nc.sync.dma_start(wv_sb[:], wvT.rearrange("(o p) f -> p o f", p=P))
            nc.sync.dma_start(wo_sb[:], woT.rearrange("(o p) e -> p o e", p=P))
            nc.sync.dma_start(dm_sb[:], dmask2[:])

            qh_sb = persist.tile([P, FT, S], BF16, tag="qh")
            kh_sb = persist.tile([P, FT, S], BF16, tag="kh")
            vh_sb = persist.tile([P, ST, H, D + 1], BF16, tag="vh")
            xts_sb = persist.tile([P, FT, S], BF16, tag="xts")

            def body():
                nc.vector.memset(vh_sb[:, :, :, D:D + 1], 1.0)

                def dma_wave(sc):
                    w = {}
                    for tag, src3 in (("xq", qT3), ("xk", kT3), ("xv", vT3)):
                        t = streams.tile([P, ET, QC], BF16, tag=tag)
                        nc.sync.dma_start(t[:], src3[:, :, sc * QC:(sc + 1) * QC])
                        w[tag] = t
                    return w

                def qk_unit(wave, which, ft, sc):
                    w_sb, dst, xc = ((wq_sb, qh_sb, wave["xq"]) if which == "q"
                                     else (wk_sb, kh_sb, wave["xk"]))
                    psum = ps_pp.tile([P, QC], F32, tag="pp")
                    for et in range(ET):
                        nc.tensor.matmul(psum[:], w_sb[:, et, ft * P:(ft + 1) * P],
                                         xc[:, et, :],
                                         start=(et == 0), stop=(et == ET - 1))
                    if qk_on_scalar and which == "q":
                        nc.scalar.copy(dst[:, ft, sc * QC:(sc + 1) * QC], psum[:])
                    else:
                        nc.vector.tensor_copy(dst[:, ft, sc * QC:(sc + 1) * QC], psum[:])

                def v_unit(wave, st):
                    si = st % KT_PER_QC
                    psum = ps_pp.tile([P, QC], F32, tag="pp")
                    for et in range(ET):
                        nc.tensor.matmul(psum[:], wave["xv"][:, et, si * P:(si + 1) * P],
                                         wv_sb[:, et, :],
                                         start=(et == 0), stop=(et == ET - 1))
                    nc.vector.tensor_copy(
                        vh_sb[:, st, :, 0:D],
                        psum[:].rearrange("p (h d) -> p h d", h=H))

                def p3_unit(jt, qc):
                    psum = ps_pp.tile([P, QC], F32, tag="pp")
                    for ft in range(FT):
                        nc.tensor.matmul(psum[:], wo_sb[:, ft, jt * P:(jt + 1) * P],
                                         xts_sb[:, ft, qc * QC:(qc + 1) * QC],
                                         start=(ft == 0), stop=(ft == FT - 1))
                    ot = otp.tile([P, QC], F32, tag="ot")
                    nc.vector.tensor_copy(ot[:], psum[:])
                    nc.sync.dma_start(
                        outT[jt * P:(jt + 1) * P, qc * QC:(qc + 1) * QC], ot[:])

                def normalize(xt_t, h, qc):
                    ft, fo = h // 2, (h % 2) * D
                    recip = smalls.tile([1, QC], F32, tag="recip")
                    if exact_recip:
                        nc.vector.reciprocal(recip[:], xt_t[D:D + 1, :])
                    else:
                        nc.vector.reciprocal_approx_fast(recip[:], xt_t[D:D + 1, :])
                    rb = smalls.tile([D, QC], F32, tag="rb")
                    nc.gpsimd.partition_broadcast(rb[:], recip[0:1, :])
                    nc.vector.tensor_mul(
                        xts_sb[fo:fo + D, ft, qc * QC:(qc + 1) * QC],
                        xt_t[0:D, :], rb[:])

                def pair_block(p, qc, wave_norm):
                    h0, h1 = 2 * p, 2 * p + 1
                    ft = p
                    nkt = 4 * qc + 4
                    xt0 = ps_xt.tile([D + 1, QC], F32, tag="xt")
                    xt1 = ps_xt.tile([D + 1, QC], F32, tag="xt")
                    pend = []

                    def flush_one():
                        kt, at_t = pend.pop(0)
                        st, sp = (kt == 0), (kt == nkt - 1)
                        jd = kt - 4 * qc
                        o = jd * P if (narrow_diag and jd > 0) else 0
                        nc.tensor.matmul(xt0[:, o:QC], vh_sb[:, kt, h0, :],
                                         at_t[:, o:QC], start=st, stop=sp)
                        nc.tensor.matmul(xt1[:, o:QC], vh_sb[:, kt, h1, :],
                                         at_t[:, QC + o:2 * QC], start=st, stop=sp)

                    for kt in range(nkt):
                        sc_t = ps_sc.tile([P, 2 * QC], F32, tag="sc")
                        for fo, cs in ((0, slice(0, QC)), (D, slice(QC, 2 * QC))):
                            nc.tensor.matmul(
                                sc_t[:, cs],
                                kh_sb[fo:fo + D, ft, kt * P:(kt + 1) * P],
                                qh_sb[fo:fo + D, ft, qc * QC:(qc + 1) * QC],
                                start=True, stop=True)
                        at_t = attnp.tile([P, 2 * QC], BF16, tag="at")
                        if split_exp:
                            for cs in (slice(0, QC), slice(QC, 2 * QC)):
                                nc.scalar.activation(at_t[:, cs], sc_t[:, cs],
                                                     mybir.ActivationFunctionType.Exp,
                                                     scale=0.125)
                        else:
                            nc.scalar.activation(at_t[:], sc_t[:],
                                                 mybir.ActivationFunctionType.Exp,
                                                 scale=0.125)
                        jd = kt - 4 * qc
                        if jd >= 0:
                            if strip_mask:
                                for half in (0, QC):
                                    s0 = half + jd * P
                                    nc.vector.tensor_mul(
                                        at_t[:, s0:s0 + P], at_t[:, s0:s0 + P],
                                        dm_sb[:, 0:P])
                                    if not narrow_diag and jd > 0:
                                        nc.vector.memset(at_t[:, half:half + jd * P], 0.0)
                            else:
                                nc.vector.tensor_mul(
                                    at_t[:], at_t[:],
                                    dm_sb[:, jd * 2 * QC:(jd + 1) * 2 * QC])
                        pend.append((kt, at_t))
                        if len(pend) > lag:
                            flush_one()
                    while pend:
                        flush_one()
                    if norm_mode == "wave":
                        for h, xt_t in ((h0, xt0), (h1, xt1)):
                            xtc = xtcp.tile([D + 1, QC], BF16, tag="xtc")
                            nc.vector.tensor_copy(xtc[:], xt_t[:])
                            wave_norm.append((h, xtc))
                    else:
                        normalize(xt0, h0, qc)
                        normalize(xt1, h1, qc)

                def wave_normalize(wave_norm, qc):
                    # Pack denominators 4-per-tile at partitions {0,32,64,96}
                    # (cross-partition moves must go through DMA) so one
                    # FD-bound reciprocal covers 4 heads.
                    for half in range(2):
                        quad = wave_norm[4 * half:4 * half + 4]
                        dnb = smalls.tile([97, QC], BF16, tag="dnb")
                        nc.vector.memset(dnb[:], 1.0)
                        for j, (h, xtc) in enumerate(quad):
                            nc.sync.dma_start(dnb[32 * j:32 * j + 1, :],
                                              xtc[D:D + 1, :])
                        dn = smalls.tile([97, QC], F32, tag="dn")
                        nc.vector.tensor_copy(dn[:], dnb[:])
                        rc = smalls.tile([97, QC], F32, tag="rc")
                        nc.vector.reciprocal(rc[:], dn[:])
                        for j, (h, xtc) in enumerate(quad):
                            ft, fo = h // 2, (h % 2) * D
                            rrow = smalls.tile([1, QC], F32, tag="rrow")
                            nc.sync.dma_start(rrow[:], rc[32 * j:32 * j + 1, :])
                            rb = smalls.tile([D, QC], F32, tag="rb")
                            nc.gpsimd.partition_broadcast(rb[:], rrow[0:1, :])
                            nc.vector.tensor_mul(
                                xts_sb[fo:fo + D, ft, qc * QC:(qc + 1) * QC],
                                xtc[0:D, :], rb[:])

                # ---- lead-in ----
                cur = dma_wave(0)
                qk_unit(cur, "q", 0, 0)
                qk_unit(cur, "k", 0, 0)
                for st in range(KT_PER_QC):
                    v_unit(cur, st)

                # ---- waves ----
                # q/k units are LAZY: emitted just before the pair that needs
                # them, so this PE work lands exactly in the ACT-paced stalls.
                for qc in range(NQC):
                    nxt = dma_wave(qc + 1) if qc + 1 < NQC else None
                    wave_norm = []
                    for p in range(FT):
                        if not (qc == 0 and p == 0):
                            qk_unit(cur, "q", p, qc)
                            qk_unit(cur, "k", p, qc)
                        pair_block(p, qc, wave_norm)
                        if nxt:
                            v_unit(nxt, 4 * (qc + 1) + p)
                        if qc >= 1:
                            p3_unit(2 * p, qc - 1)
                            p3_unit(2 * p + 1, qc - 1)
                    if norm_mode == "wave":
                        wave_normalize(wave_norm, qc)
                    cur = nxt

                # ---- tail ----
                for jt in range(ET):
                    p3_unit(jt, NQC - 1)

            if niter is None:
                body()
            else:
                with tc.For_i(0, niter, 1):
                    body()

    nc.compile()
    return nc


def _host_prep(q, k, v, mask, w_q, w_k, w_v, w_o):
    """Shard + transpose inputs on the host.  Returns (in_maps, causal)."""
    tril = np.tril(np.ones((S, S), dtype=mask.dtype))
    causal = all(np.array_equal(np.asarray(mask[b, 0]), tril) for b in range(B))

    stair = (np.arange(2 * QC)[None, :] >= (np.arange(P)[:, None] + QC))
    stair = stair.astype(NPBF16)

    # v2 diag mask: 4 concatenated [P, QC] tiles, tile jd valid iff q >= p + 128*jd
    dmask = np.concatenate(
        [(np.arange(QC)[None, :] >= (np.arange(P)[:, None] + P * jd))
         for jd in range(4)], axis=1).astype(NPBF16)
    # v3 pair mask: same, but each jd tile duplicated side-by-side for the
    # two heads of a pair sharing one [P, 2*QC] at-tile
    dmask2 = np.concatenate(
        [np.tile((np.arange(QC)[None, :] >= (np.arange(P)[:, None] + P * jd)), (1, 2))
         for jd in range(4)], axis=1).astype(NPBF16)

    w_q = np.asarray(w_q, dtype=np.float32)
    w_k = np.asarray(w_k, dtype=np.float32)
    w_v = np.asarray(w_v, dtype=np.float32)
    w_o = np.asarray(w_o, dtype=np.float32)

    in_maps = []
    for core in range(8):
        b, g = divmod(core, 2)
        rows = slice(g * F, (g + 1) * F)
        m = {
            "qT": np.ascontiguousarray(np.asarray(q[b], np.float32).T).astype(NPBF16),
            "kT": np.ascontiguousarray(np.asarray(k[b], np.float32).T).astype(NPBF16),
            "vT": np.ascontiguousarray(np.asarray(v[b], np.float32).T).astype(NPBF16),
            "wqT": np.ascontiguousarray(w_q[rows, :].T).astype(NPBF16),
            "wkT": np.ascontiguousarray(w_k[rows, :].T).astype(NPBF16),
            "wvT": np.ascontiguousarray(w_v[rows, :].T).astype(NPBF16),
            "woT": np.ascontiguousarray(w_o[:, rows].T).astype(NPBF16),
            "stair": stair,
            "dmask": dmask,
            "dmask2": dmask2,
        }
        if not causal:
            m["maskT"] = np.ascontiguousarray(
                np.asarray(mask[b, 0], np.float32).T).astype(NPBF16)
        in_maps.append(m)
    return in_maps, causal


_NC_CACHE: dict = {}


def kernel(q, k, v, mask, w_q, w_k, w_v, w_o):
    in_maps, causal = _host_prep(q, k, v, mask, w_q, w_k, w_v, w_o)
    nc = _NC_CACHE.get(causal)
    if nc is None:
        nc = build_nc3() if causal else build_nc(causal)
        _NC_CACHE[causal] = nc
    res = bass_utils.run_bass_kernel_spmd(nc, in_maps, core_ids=list(range(8)))
    out = np.empty((B, S, E), dtype=np.float32)
    for b in range(B):
        out[b] = (res.results[2 * b]["outT"] + res.results[2 * b + 1]["outT"]).T
    return out

